# revision 17
# baseline (speedup 1.0000x reference)
"""Trainium2 Bass kernel for nn_AttentionPatch (patch attention block).

Reference computation (per batch b, group g):
    qkv  = w_qkv @ x[b,:,g,:]            # [2304, 256] channel matmul
    q,k,v per head (12 heads, hd=64)
    S    = (q^T k) * hd**-0.5            # [256 query, 256 key]
    P    = exp(S) * mask[g]              # masked softmax numerator
    att  = (P @ v) / rowsum(P)
    out  = w_proj @ att + b_proj

Sharding: data-parallel over the 64 groups (8 per core), zero communication.

Layout strategy (all matmuls keep channels on partitions, tokens on free):
    x_sbuf  [128d x 6, 512]  two batches side by side (N=512 moving operand)
    q,k     [hd, token] from QKV matmul;  v produced directly as [token, hd]
            (by swapping stationary/moving) so attention needs no transposes.
    S^T     [key, query] via lhsT=k_slice, rhs=q  (contraction over hd=64)
    P^T     = exp(S^T) * maskT  on ACT/DVE; mask transposed on host.
    AV      lhsT=[v|ones] [key,65], rhs=P^T -> [hd|rowsum, query]; the ones
            column yields the softmax denominator for free.
    norm    recip(rowsum) broadcast across partitions with a rank-1 matmul.
    proj    lhsT=w_projT chunks, rhs=normalized att [d, token].

All matmuls run as float32r (full PE rate at moving dim >= 256).
"""

import numpy as np

import concourse.bass as bass
import concourse.tile as tile
from concourse import mybir
from concourse.bass_utils import run_bass_kernel_spmd

# The walrus build in this container rejects instructions carrying more
# than ~2 semaphore waits ("Too many sync wait commands" in setupSyncWait).
# TileContext's kernel-tail drain accumulates one wait per live semaphore
# on a single SP drain, which trips that limit for any nontrivial kernel.
# Split those waits across single-wait SP nops ahead of the drain.
def _patched_drain_and_barrier(self, tick_clock, wait_clock):
    probe = self.nc.sync.nop(nofuse=True, hint="tile_tail_waits")
    wait_clock.add_sem_waits(
        probe.ins, tile.ScopedClock({None: tick_clock.global_clock})
    )
    waits = list(probe.ins.sync_info.on_wait or [])
    probe.ins.sync_info.on_wait = waits[:1]
    import bass_rust as _br

    for w in waits[1:]:
        ni = self.nc.sync.nop(nofuse=True, hint="tile_tail_waits")
        ni.ins.sync_info = _br.SyncInfo(on_wait=[w], on_update=[])
    self.nc.sync.drain()

    self.nc.all_engine_barrier()
    assert self.sems is not None
    popped = self.nc._tile_sem_poison_stack.pop()
    assert popped is self._sem_poison
    self.nc.clear_and_free_semaphores(list(self.sems.allocated().values()))
    self.nc.all_engine_barrier()


tile.TileContext._drain_and_barrier = _patched_drain_and_barrier


# Same walrus limit, applied generally: any instruction carrying more sem
# waits than the ISA sync field supports is rejected at codegen. Move the
# excess onto same-engine nops placed immediately before the instruction
# (identical semantics: the engine blocks on the nop's waits first).
def _split_excess_waits(nc, max_waits=1):
    import bass_rust as _br

    def make_nop(engine):
        ins = nc.engines[engine].nop(hint="wait_split", nofuse=True).ins
        for bb in nc.m.functions[0].blocks:
            lst = bb.instructions
            if lst and lst[-1] is ins:
                lst.pop()
        return ins

    for bb in nc.m.functions[0].blocks:
        insts = bb.instructions
        i = 0
        while i < len(insts):
            inst = insts[i]
            si = inst.sync_info
            waits = list(si.on_wait) if si and si.on_wait else []
            if len(waits) > max_waits:
                extras = waits[: len(waits) - max_waits]
                new_nops = []
                for j in range(0, len(extras), max_waits):
                    nop_inst = make_nop(inst.engine)
                    nop_inst.sync_info = _br.SyncInfo(
                        on_wait=extras[j:j + max_waits], on_update=[]
                    )
                    new_nops.append(nop_inst)
                si.on_wait = waits[len(waits) - max_waits:]
                insts[i:i] = new_nops
                i += len(new_nops)
            i += 1

B, D, G, P = 4, 768, 64, 256
H, HD = 12, 64
SCALE = HD ** -0.5
N_CORES = 8
GC = G // N_CORES  # groups per core
DC = D // 128      # 128-partition chunks of the channel dim
F32 = mybir.dt.float32
F32R = mybir.dt.float32r

AF = mybir.ActivationFunctionType


def build_nc():
    nc = bass.Bass("TRN2")
    x_d = nc.dram_tensor("x", [B, D, GC, P], F32R, kind="ExternalInput")
    mt_d = nc.dram_tensor("maskT", [GC, P, P], F32, kind="ExternalInput")
    wq_d = nc.dram_tensor("wqkvT", [D, 3 * D], F32R, kind="ExternalInput")
    wp_d = nc.dram_tensor("wprojT", [D, D], F32R, kind="ExternalInput")
    bp_d = nc.dram_tensor("bproj", [D], F32, kind="ExternalInput")
    on_d = nc.dram_tensor("ones_c", [128, 64], F32R, kind="ExternalInput")
    pc_d = nc.dram_tensor("pair_c", [33, 128], F32R, kind="ExternalInput")
    rz_d = nc.dram_tensor("rzero", [31, P], F32R, kind="ExternalInput")
    o_d = nc.dram_tensor("out", [B, D, GC, P], F32, kind="ExternalOutput")

    with tile.TileContext(nc) as tc, nc.allow_low_precision(
        reason="float32r tiles feed full-rate PE matmuls; data is fp32"
    ):
        with (
            tc.tile_pool(name="wpool", bufs=1) as wpool,
            tc.tile_pool(name="maskp", bufs=2) as maskp,
            tc.tile_pool(name="xp", bufs=2) as xp,
            tc.tile_pool(name="qkvp", bufs=1) as qkvp,
            tc.tile_pool(name="ptp", bufs=4) as ptp,
            tc.tile_pool(name="attp", bufs=2) as attp,
            tc.tile_pool(name="pjp", bufs=2) as pjp,
            tc.tile_pool(name="otp", bufs=3) as otp,
            tc.tile_pool(name="psa", bufs=4, space="PSUM") as psa,
            tc.tile_pool(name="psb", bufs=4, space="PSUM") as psb,
        ):
            wq = wpool.tile([128, DC, 3 * D], F32R)
            for dc in range(DC):
                nc.sync.dma_start(out=wq[:, dc, :], in_=wq_d[128 * dc:128 * (dc + 1), :])
            wp = wpool.tile([128, DC, D], F32R)
            for dc in range(DC):
                nc.sync.dma_start(out=wp[:, dc, :], in_=wp_d[128 * dc:128 * (dc + 1), :])
            bias = wpool.tile([128, DC], F32)
            nc.sync.dma_start(out=bias, in_=bp_d[:].rearrange("(c p) -> p c", p=128))
            pairc = wpool.tile([33, 128], F32R)
            nc.sync.dma_start(out=pairc, in_=pc_d[:, :])
            rshs = []
            for ri in range(4):
                rt = wpool.tile([33, P], F32R, name=f"rsh{ri}", tag=f"rsh{ri}")
                nc.sync.dma_start(out=rt[1:32, :], in_=rz_d[:, :])
                rshs.append(rt)
            rsh_i = 0
            vt = wpool.tile([128, 2, 2, H, HD + 1], F32R)
            nc.sync.dma_start(
                out=vt[:, :, :, :, HD],
                in_=on_d[:, 0:48].rearrange("p (a b h) -> p a b h", a=2, b=2),
            )

            for gi in range(GC):
                mk = maskp.tile([128, 2, P], F32, tag="mk")
                for kc in range(2):
                    nc.sync.dma_start(
                        out=mk[:, kc, :], in_=mt_d[gi, 128 * kc:128 * (kc + 1), :]
                    )
                for bp2 in range(2):  # batch pairs
                    xt = xp.tile([128, DC, 2 * P], F32R, tag="xt")
                    for b2 in range(2):
                        b = 2 * bp2 + b2
                        for dc in range(DC):
                            nc.sync.dma_start(
                                out=xt[:, dc, P * b2:P * (b2 + 1)],
                                in_=x_d[b, 128 * dc:128 * (dc + 1), gi, :],
                            )
                    qt = qkvp.tile([128, DC, 2 * P], F32R, tag="qt")
                    kt = qkvp.tile([128, DC, 2 * P], F32R, tag="kt")
                    # q,k projection: [e, token], both batches as N=512 moving
                    for c in range(2 * DC):
                        ps = psa.tile([128, 2 * P], F32, tag="psa")
                        for dc in range(DC):
                            nc.tensor.matmul(
                                ps,
                                wq[:, dc, 128 * c:128 * (c + 1)],
                                xt[:, dc, :],
                                start=(dc == 0),
                                stop=(dc == DC - 1),
                            )
                        dst = qt if c < DC else kt
                        nc.scalar.copy(dst[:, c % DC, :], ps)
                    # v projection, transposed: [token, e] (x chunks stationary)
                    for b2 in range(2):
                        for tkc in range(2):
                            for nh in range(2):
                                psv = psa.tile([128, 384], F32, tag="psa")
                                t0 = P * b2 + 128 * tkc
                                for dc in range(DC):
                                    nc.tensor.matmul(
                                        psv,
                                        xt[:, dc, t0:t0 + 128],
                                        wq[:, dc, 2 * D + 384 * nh:2 * D + 384 * (nh + 1)],
                                        start=(dc == 0),
                                        stop=(dc == DC - 1),
                                    )
                                nc.scalar.copy(
                                    vt[:, b2, tkc, 6 * nh:6 * (nh + 1), 0:HD],
                                    psv[:, :].rearrange("p (h d) -> p h d", h=6),
                                )

                    pjt = pjp.tile([128, DC, 2 * P], F32R, tag="pjt")
                    for b2 in range(2):
                        at = attp.tile([128, DC, P], F32, tag="at")
                        for c in range(DC):
                            rsh = rshs[rsh_i % 4]
                            rsh_i += 1
                            for half in range(2):
                                h = 2 * c + half
                                r0 = 64 * half
                                sps = psa.tile([128, 2 * P], F32, tag="psa")
                                pts = []
                                for kc in range(2):
                                    nc.tensor.matmul(
                                        sps[:, P * kc:P * (kc + 1)],
                                        kt[r0:r0 + 64, c, P * b2 + 128 * kc:P * b2 + 128 * (kc + 1)],
                                        qt[r0:r0 + 64, c, P * b2:P * (b2 + 1)],
                                        start=True,
                                        stop=True,
                                    )
                                    pt = ptp.tile([128, P], F32R, tag="pt")
                                    nc.scalar.activation(
                                        pt, sps[:, P * kc:P * (kc + 1)], AF.Exp, scale=SCALE
                                    )
                                    nc.vector.tensor_mul(pt, pt, mk[:, kc, :])
                                    pts.append(pt)
                                ops = psb.tile([HD + 1, P], F32, tag="psb")
                                for kc in range(2):
                                    nc.tensor.matmul(
                                        ops,
                                        vt[:, b2, kc, h, :],
                                        pts[kc],
                                        start=(kc == 0),
                                        stop=(kc == 1),
                                    )
                                # softmax denominators for the pair land on
                                # adjacent partitions 64 (even) / 65 (odd)
                                nc.vector.reciprocal(
                                    rsh[32 * half:32 * half + 1, :], ops[HD:HD + 1, :]
                                )
                                nc.scalar.copy(at[r0:r0 + 64, c, :], ops[0:HD, :])
                            # broadcast both recips across partitions in one
                            # rank-2 matmul: rows 0-63 <- recip_even, 64-127
                            # <- recip_odd (matmul output must start at
                            # partition 0, so odd heads can't get their own)
                            Rps = psb.tile([128, P], F32, tag="psb")
                            nc.tensor.matmul(
                                Rps,
                                pairc,
                                rsh[0:33, :],
                                start=True,
                                stop=True,
                            )
                            for half in range(2):
                                r0 = 64 * half
                                nc.vector.tensor_mul(
                                    pjt[r0:r0 + 64, c, P * b2:P * (b2 + 1)],
                                    at[r0:r0 + 64, c, :],
                                    Rps[r0:r0 + 64, :],
                                )
                    # output projection, both batches as N=512 moving
                    for ec in range(DC):
                        pps = psa.tile([128, 2 * P], F32, tag="psa")
                        for dc in range(DC):
                            nc.tensor.matmul(
                                pps,
                                wp[:, dc, 128 * ec:128 * (ec + 1)],
                                pjt[:, dc, :],
                                start=(dc == 0),
                                stop=(dc == DC - 1),
                            )
                        ot = otp.tile([128, 2 * P], F32, tag="ot")
                        nc.vector.tensor_scalar_add(ot, pps, bias[:, ec:ec + 1])
                        for b2 in range(2):
                            nc.sync.dma_start(
                                out=o_d[2 * bp2 + b2, 128 * ec:128 * (ec + 1), gi, :],
                                in_=ot[:, P * b2:P * (b2 + 1)],
                            )
    _split_excess_waits(nc)
    return nc


def _prep_in_maps(x, mask, w_qkv, w_proj, b_proj):
    x = np.ascontiguousarray(x, dtype=np.float32)
    maskT = np.ascontiguousarray(
        np.transpose(mask, (0, 2, 1)).astype(np.float32)
    )  # [g, key, query]
    wqkvT = np.ascontiguousarray(w_qkv.T.astype(np.float32))
    wprojT = np.ascontiguousarray(w_proj.T.astype(np.float32))
    b_proj = np.ascontiguousarray(b_proj, dtype=np.float32)
    pair_c = np.zeros((33, 128), dtype=np.float32)
    pair_c[0, 0:64] = 1.0
    pair_c[32, 64:128] = 1.0
    in_maps = []
    for i in range(N_CORES):
        gs = slice(i * GC, (i + 1) * GC)
        in_maps.append(
            {
                "x": np.ascontiguousarray(x[:, :, gs, :]),
                "maskT": np.ascontiguousarray(maskT[gs]),
                "wqkvT": wqkvT,
                "wprojT": wprojT,
                "bproj": b_proj,
                "ones_c": np.ones((128, 64), dtype=np.float32),
                "pair_c": pair_c,
                "rzero": np.zeros((31, P), dtype=np.float32),
            }
        )
    return in_maps


def _run(inputs, trace=False):
    nc = build_nc()
    in_maps = _prep_in_maps(
        inputs["x"], inputs["mask"], inputs["w_qkv"], inputs["w_proj"], inputs["b_proj"]
    )
    res = run_bass_kernel_spmd(nc, in_maps, list(range(N_CORES)), trace=trace)
    out = np.concatenate([res.results[i]["out"] for i in range(N_CORES)], axis=2)
    return out.astype(np.float32), res


def kernel(x, mask, w_qkv, w_proj, b_proj):
    out, _ = _run(
        {"x": x, "mask": mask, "w_qkv": w_qkv, "w_proj": w_proj, "b_proj": b_proj}
    )
    return out


# revision 18
# speedup vs baseline: 1.3442x; 1.3442x over previous
"""Trainium2 Bass kernel for nn_AttentionPatch (patch attention block).

Reference computation (per batch b, group g):
    qkv  = w_qkv @ x[b,:,g,:]            # [2304, 256] channel matmul
    q,k,v per head (12 heads, hd=64)
    S    = (q^T k) * hd**-0.5            # [256 query, 256 key]
    P    = exp(S) * mask[g]              # masked softmax numerator
    att  = (P @ v) / rowsum(P)
    out  = w_proj @ att + b_proj

Sharding: data-parallel over the 64 groups (8 per core), zero communication.

Layout strategy (all matmuls keep channels on partitions, tokens on free):
    x_sbuf  [128d x 6, 512]  two batches side by side (N=512 moving operand)
    q,k     [hd, token] from QKV matmul;  v produced directly as [token, hd]
            (by swapping stationary/moving) so attention needs no transposes.
    S^T     [key, query] via lhsT=k_slice, rhs=q  (contraction over hd=64)
    P^T     = exp(S^T) * maskT in bf16 (ACT+DVE); mask transposed on host.
    AV      lhsT=[v|ones] [key,65] bf16, rhs=P^T -> [hd|rowsum, query]; the
            ones column yields the softmax denominator for free.
    norm    rowsums of 4 heads gathered at partitions {0,32,64,96} of one
            tile, one batched reciprocal, then per-head-pair rank-2 matmuls
            broadcast the recips across partitions (deferred to the end of
            each batch so the PE never stalls behind a reciprocal).
    proj    lhsT=w_projT chunks, rhs=normalized att [d, token].

QKV / S / proj matmuls run as float32r (full PE rate at moving dim >= 256,
~1e-4 relative error); the attention-probability path runs in bf16.
"""

import numpy as np
import ml_dtypes

import concourse.bass as bass
import concourse.tile as tile
from concourse import mybir
from concourse.bass_utils import run_bass_kernel_spmd

# The walrus build in this container rejects instructions carrying more
# sem waits than the ISA sync field supports ("Too many sync wait
# commands" in setupSyncWait). TileContext's kernel-tail drain accumulates
# one wait per live semaphore on a single SP drain, which trips that
# limit for any nontrivial kernel. Split across single-wait SP nops.
def _patched_drain_and_barrier(self, tick_clock, wait_clock):
    probe = self.nc.sync.nop(nofuse=True, hint="tile_tail_waits")
    wait_clock.add_sem_waits(
        probe.ins, tile.ScopedClock({None: tick_clock.global_clock})
    )
    waits = list(probe.ins.sync_info.on_wait or [])
    probe.ins.sync_info.on_wait = waits[:1]
    import bass_rust as _br

    for w in waits[1:]:
        ni = self.nc.sync.nop(nofuse=True, hint="tile_tail_waits")
        ni.ins.sync_info = _br.SyncInfo(on_wait=[w], on_update=[])
    self.nc.sync.drain()

    self.nc.all_engine_barrier()
    assert self.sems is not None
    popped = self.nc._tile_sem_poison_stack.pop()
    assert popped is self._sem_poison
    self.nc.clear_and_free_semaphores(list(self.sems.allocated().values()))
    self.nc.all_engine_barrier()


tile.TileContext._drain_and_barrier = _patched_drain_and_barrier


# Same walrus limit, applied generally: move excess waits onto same-engine
# nops placed immediately before the instruction (identical semantics: the
# engine blocks on the nop's waits first).
def _split_excess_waits(nc, max_waits=1):
    import bass_rust as _br

    def make_nop(engine):
        ins = nc.engines[engine].nop(hint="wait_split", nofuse=True).ins
        for bb in nc.m.functions[0].blocks:
            lst = bb.instructions
            if lst and lst[-1] is ins:
                lst.pop()
        return ins

    for bb in nc.m.functions[0].blocks:
        insts = bb.instructions
        i = 0
        while i < len(insts):
            inst = insts[i]
            si = inst.sync_info
            waits = list(si.on_wait) if si and si.on_wait else []
            if len(waits) > max_waits:
                extras = waits[: len(waits) - max_waits]
                new_nops = []
                for j in range(0, len(extras), max_waits):
                    nop_inst = make_nop(inst.engine)
                    nop_inst.sync_info = _br.SyncInfo(
                        on_wait=extras[j:j + max_waits], on_update=[]
                    )
                    new_nops.append(nop_inst)
                si.on_wait = waits[len(waits) - max_waits:]
                insts[i:i] = new_nops
                i += len(new_nops)
            i += 1


B, D, G, P = 4, 768, 64, 256
H, HD = 12, 64
SCALE = HD ** -0.5
N_CORES = 8
GC = G // N_CORES  # groups per core
DC = D // 128      # 128-partition chunks of the channel dim
F32 = mybir.dt.float32
F32R = mybir.dt.float32r
BF16 = mybir.dt.bfloat16

AF = mybir.ActivationFunctionType


def build_nc():
    nc = bass.Bass("TRN2")
    x_d = nc.dram_tensor("x", [B, D, GC, P], F32R, kind="ExternalInput")
    mt_d = nc.dram_tensor("maskT", [GC, P, P], BF16, kind="ExternalInput")
    wq_d = nc.dram_tensor("wqkvT", [D, 3 * D], F32R, kind="ExternalInput")
    wp_d = nc.dram_tensor("wprojT", [D, D], F32R, kind="ExternalInput")
    bp_d = nc.dram_tensor("bproj", [D], F32, kind="ExternalInput")
    on_d = nc.dram_tensor("ones_c", [128, 64], BF16, kind="ExternalInput")
    pc_d = nc.dram_tensor("pair_c", [97, 128], F32R, kind="ExternalInput")
    rz_d = nc.dram_tensor("rones", [97, P], F32R, kind="ExternalInput")
    o_d = nc.dram_tensor("out", [B, D, GC, P], F32, kind="ExternalOutput")

    with tile.TileContext(nc) as tc, nc.allow_low_precision(
        reason="float32r/bf16 matmul inputs; accumulation stays fp32"
    ):
        with (
            tc.tile_pool(name="wpool", bufs=1) as wpool,
            tc.tile_pool(name="maskp", bufs=2) as maskp,
            tc.tile_pool(name="xp", bufs=2) as xp,
            tc.tile_pool(name="qkvp", bufs=1) as qkvp,
            tc.tile_pool(name="ptp", bufs=4) as ptp,
            tc.tile_pool(name="attp", bufs=1) as attp,
            tc.tile_pool(name="pjp", bufs=1) as pjp,
            tc.tile_pool(name="otp", bufs=2) as otp,
            tc.tile_pool(name="psa", bufs=4, space="PSUM") as psa,
            tc.tile_pool(name="psb", bufs=4, space="PSUM") as psb,
        ):
            wq = wpool.tile([128, DC, 3 * D], F32R)
            for dc in range(DC):
                nc.sync.dma_start(out=wq[:, dc, :], in_=wq_d[128 * dc:128 * (dc + 1), :])
            wp = wpool.tile([128, DC, D], F32R)
            for dc in range(DC):
                nc.sync.dma_start(out=wp[:, dc, :], in_=wp_d[128 * dc:128 * (dc + 1), :])
            bias = wpool.tile([128, DC], F32)
            nc.sync.dma_start(out=bias, in_=bp_d[:].rearrange("(c p) -> p c", p=128))
            pairc = wpool.tile([97, 128], F32R)
            nc.sync.dma_start(out=pairc, in_=pc_d[:, :])
            # rowsum gather tiles: 4 heads' denominators per tile at
            # partitions {0,32,64,96} (engine writes must be 32-aligned);
            # in-between rows are kept at 1.0 (ones-init, and 1/1 stays 1
            # through the in-place reciprocal) so the zero rows of pair_c
            # never meet Inf/NaN.
            gts = []
            for ri in range(6):
                gt = wpool.tile([97, P], F32R, name=f"gt{ri}", tag=f"gt{ri}")
                nc.sync.dma_start(out=gt, in_=rz_d[:, :])
                gts.append(gt)
            gt_i = 0
            vt = wpool.tile([128, 2, 2, H, HD + 1], BF16)
            nc.sync.dma_start(
                out=vt[:, :, :, :, HD],
                in_=on_d[:, 0:48].rearrange("p (a b h) -> p a b h", a=2, b=2),
            )

            for gi in range(GC):
                mk = maskp.tile([128, 2, P], BF16, tag="mk")
                for kc in range(2):
                    nc.sync.dma_start(
                        out=mk[:, kc, :], in_=mt_d[gi, 128 * kc:128 * (kc + 1), :]
                    )
                for bp2 in range(2):  # batch pairs
                    xt = xp.tile([128, DC, 2 * P], F32R, tag="xt")
                    for b2 in range(2):
                        b = 2 * bp2 + b2
                        for dc in range(DC):
                            nc.sync.dma_start(
                                out=xt[:, dc, P * b2:P * (b2 + 1)],
                                in_=x_d[b, 128 * dc:128 * (dc + 1), gi, :],
                            )
                    qt = qkvp.tile([128, DC, 2 * P], F32R, tag="qt")
                    kt = qkvp.tile([128, DC, 2 * P], F32R, tag="kt")
                    # q,k projection: [e, token], both batches as N=512 moving
                    for c in range(2 * DC):
                        ps = psa.tile([128, 2 * P], F32, tag="psa")
                        for dc in range(DC):
                            nc.tensor.matmul(
                                ps,
                                wq[:, dc, 128 * c:128 * (c + 1)],
                                xt[:, dc, :],
                                start=(dc == 0),
                                stop=(dc == DC - 1),
                            )
                        dst = qt if c < DC else kt
                        nc.scalar.copy(dst[:, c % DC, :], ps)
                    # v projection, transposed: [token, e] (x chunks stationary)
                    for b2 in range(2):
                        for tkc in range(2):
                            for nh in range(2):
                                psv = psa.tile([128, 384], F32, tag="psa")
                                t0 = P * b2 + 128 * tkc
                                for dc in range(DC):
                                    nc.tensor.matmul(
                                        psv,
                                        xt[:, dc, t0:t0 + 128],
                                        wq[:, dc, 2 * D + 384 * nh:2 * D + 384 * (nh + 1)],
                                        start=(dc == 0),
                                        stop=(dc == DC - 1),
                                    )
                                nc.scalar.copy(
                                    vt[:, b2, tkc, 6 * nh:6 * (nh + 1), 0:HD],
                                    psv[:, :].rearrange("p (h d) -> p h d", h=6),
                                )

                    pjt = pjp.tile([128, DC, 2 * P], F32R, tag="pjt")
                    for b2 in range(2):
                        at = attp.tile([128, DC, P], F32, tag="at")
                        grp_tiles = [gts[(gt_i + j) % 6] for j in range(3)]
                        gt_i += 3
                        for h in range(H):
                            c, half = divmod(h, 2)
                            r0 = 64 * half
                            gt = grp_tiles[h // 4]
                            grow = 32 * (h % 4)
                            sps = psa.tile([128, 2 * P], F32, tag="psa")
                            for kc in range(2):
                                nc.tensor.matmul(
                                    sps[:, P * kc:P * (kc + 1)],
                                    kt[r0:r0 + 64, c, P * b2 + 128 * kc:P * b2 + 128 * (kc + 1)],
                                    qt[r0:r0 + 64, c, P * b2:P * (b2 + 1)],
                                    start=True,
                                    stop=True,
                                )
                            pt = ptp.tile([128, 2 * P], BF16, tag="pt")
                            nc.scalar.activation(pt, sps, AF.Exp, scale=SCALE)
                            nc.vector.tensor_mul(
                                pt, pt, mk[:, :, :].rearrange("p a q -> p (a q)")
                            )
                            ops = psb.tile([HD + 1, P], F32, tag="psb")
                            for kc in range(2):
                                nc.tensor.matmul(
                                    ops,
                                    vt[:, b2, kc, h, :],
                                    pt[:, P * kc:P * (kc + 1)],
                                    start=(kc == 0),
                                    stop=(kc == 1),
                                )
                            nc.scalar.copy(gt[grow:grow + 1, :], ops[HD:HD + 1, :])
                            nc.vector.tensor_copy(at[r0:r0 + 64, c, :], ops[0:HD, :])
                        # batched softmax denominators + partition broadcast,
                        # off the per-head critical path
                        for grp in range(3):
                            gt = grp_tiles[grp]
                            nc.vector.reciprocal(gt[:, :], gt[:, :])
                            for pr in range(2):
                                Rps = psb.tile([128, P], F32, tag="psb")
                                nc.tensor.matmul(
                                    Rps,
                                    pairc[64 * pr:64 * pr + 33, :],
                                    gt[64 * pr:64 * pr + 33, :],
                                    start=True,
                                    stop=True,
                                )
                                for half in range(2):
                                    h = 4 * grp + 2 * pr + half
                                    c = h // 2
                                    r0 = 64 * half
                                    nc.vector.tensor_mul(
                                        pjt[r0:r0 + 64, c, P * b2:P * (b2 + 1)],
                                        at[r0:r0 + 64, c, :],
                                        Rps[r0:r0 + 64, :],
                                    )
                    # output projection, both batches as N=512 moving
                    for ec in range(DC):
                        pps = psa.tile([128, 2 * P], F32, tag="psa")
                        for dc in range(DC):
                            nc.tensor.matmul(
                                pps,
                                wp[:, dc, 128 * ec:128 * (ec + 1)],
                                pjt[:, dc, :],
                                start=(dc == 0),
                                stop=(dc == DC - 1),
                            )
                        ot = otp.tile([128, 2 * P], F32, tag="ot")
                        nc.vector.tensor_scalar_add(ot, pps, bias[:, ec:ec + 1])
                        for b2 in range(2):
                            nc.sync.dma_start(
                                out=o_d[2 * bp2 + b2, 128 * ec:128 * (ec + 1), gi, :],
                                in_=ot[:, P * b2:P * (b2 + 1)],
                            )
    _split_excess_waits(nc)
    return nc


def _prep_in_maps(x, mask, w_qkv, w_proj, b_proj):
    x = np.ascontiguousarray(x, dtype=np.float32)
    maskT = np.ascontiguousarray(
        np.transpose(mask, (0, 2, 1)).astype(ml_dtypes.bfloat16)
    )  # [g, key, query], exact 0/1 in bf16
    wqkvT = np.ascontiguousarray(w_qkv.T.astype(np.float32))
    wprojT = np.ascontiguousarray(w_proj.T.astype(np.float32))
    b_proj = np.ascontiguousarray(b_proj, dtype=np.float32)
    pair_c = np.zeros((97, 128), dtype=np.float32)
    pair_c[0, 0:64] = 1.0
    pair_c[32, 64:128] = 1.0
    pair_c[64, 0:64] = 1.0
    pair_c[96, 64:128] = 1.0
    in_maps = []
    for i in range(N_CORES):
        gs = slice(i * GC, (i + 1) * GC)
        in_maps.append(
            {
                "x": np.ascontiguousarray(x[:, :, gs, :]),
                "maskT": np.ascontiguousarray(maskT[gs]),
                "wqkvT": wqkvT,
                "wprojT": wprojT,
                "bproj": b_proj,
                "ones_c": np.ones((128, 64), dtype=ml_dtypes.bfloat16),
                "pair_c": pair_c,
                "rones": np.ones((97, P), dtype=np.float32),
            }
        )
    return in_maps


def _run(inputs, trace=False):
    nc = build_nc()
    in_maps = _prep_in_maps(
        inputs["x"], inputs["mask"], inputs["w_qkv"], inputs["w_proj"], inputs["b_proj"]
    )
    res = run_bass_kernel_spmd(nc, in_maps, list(range(N_CORES)), trace=trace)
    out = np.concatenate([res.results[i]["out"] for i in range(N_CORES)], axis=2)
    return out.astype(np.float32), res


def kernel(x, mask, w_qkv, w_proj, b_proj):
    out, _ = _run(
        {"x": x, "mask": mask, "w_qkv": w_qkv, "w_proj": w_proj, "b_proj": b_proj}
    )
    return out


# revision 19
# speedup vs baseline: 1.3653x; 1.0157x over previous
"""Trainium2 Bass kernel for nn_AttentionPatch (patch attention block).

Reference computation (per batch b, group g):
    qkv  = w_qkv @ x[b,:,g,:]            # [2304, 256] channel matmul
    q,k,v per head (12 heads, hd=64)
    S    = (q^T k) * hd**-0.5            # [256 query, 256 key]
    P    = exp(S) * mask[g]              # masked softmax numerator
    att  = (P @ v) / rowsum(P)
    out  = w_proj @ att + b_proj

Sharding: data-parallel over the 64 groups (8 per core), zero communication.

Layout strategy (all matmuls keep channels on partitions, tokens on free):
    x_sbuf  [128d x 6, 512]  two batches side by side (N=512 moving operand)
    q,k     [hd, token] from QKV matmul;  v produced directly as [token, hd]
            (by swapping stationary/moving) so attention needs no transposes.
    S^T     [key, query] via lhsT=k_slice, rhs=q  (contraction over hd=64)
    P^T     = exp(S^T) * maskT in bf16 (ACT+DVE); mask transposed on host.
    AV      lhsT=[v|ones] [key,65] bf16, rhs=P^T -> [hd|rowsum, query]; the
            ones column yields the softmax denominator for free.
    norm    rowsums of 4 heads gathered at partitions {0,32,64,96} of one
            tile, one batched reciprocal, then per-head-pair rank-2 matmuls
            broadcast the recips across partitions (deferred to the end of
            each batch so the PE never stalls behind a reciprocal).
    proj    lhsT=w_projT chunks, rhs=normalized att [d, token].

QKV / S / proj matmuls run as float32r (full PE rate at moving dim >= 256,
~1e-4 relative error); the attention-probability path runs in bf16.
"""

import numpy as np
import ml_dtypes

import concourse.bass as bass
import concourse.tile as tile
from concourse import mybir
from concourse.bass_utils import run_bass_kernel_spmd

# The walrus build in this container rejects instructions carrying more
# sem waits than the ISA sync field supports ("Too many sync wait
# commands" in setupSyncWait). TileContext's kernel-tail drain accumulates
# one wait per live semaphore on a single SP drain, which trips that
# limit for any nontrivial kernel. Split across single-wait SP nops.
def _patched_drain_and_barrier(self, tick_clock, wait_clock):
    probe = self.nc.sync.nop(nofuse=True, hint="tile_tail_waits")
    wait_clock.add_sem_waits(
        probe.ins, tile.ScopedClock({None: tick_clock.global_clock})
    )
    waits = list(probe.ins.sync_info.on_wait or [])
    probe.ins.sync_info.on_wait = waits[:1]
    import bass_rust as _br

    for w in waits[1:]:
        ni = self.nc.sync.nop(nofuse=True, hint="tile_tail_waits")
        ni.ins.sync_info = _br.SyncInfo(on_wait=[w], on_update=[])
    self.nc.sync.drain()

    self.nc.all_engine_barrier()
    assert self.sems is not None
    popped = self.nc._tile_sem_poison_stack.pop()
    assert popped is self._sem_poison
    self.nc.clear_and_free_semaphores(list(self.sems.allocated().values()))
    self.nc.all_engine_barrier()


tile.TileContext._drain_and_barrier = _patched_drain_and_barrier


# Same walrus limit, applied generally: move excess waits onto same-engine
# nops placed immediately before the instruction (identical semantics: the
# engine blocks on the nop's waits first).
def _split_excess_waits(nc, max_waits=1):
    import bass_rust as _br

    def make_nop(engine):
        ins = nc.engines[engine].nop(hint="wait_split", nofuse=True).ins
        for bb in nc.m.functions[0].blocks:
            lst = bb.instructions
            if lst and lst[-1] is ins:
                lst.pop()
        return ins

    for bb in nc.m.functions[0].blocks:
        insts = bb.instructions
        i = 0
        while i < len(insts):
            inst = insts[i]
            si = inst.sync_info
            waits = list(si.on_wait) if si and si.on_wait else []
            if len(waits) > max_waits:
                extras = waits[: len(waits) - max_waits]
                new_nops = []
                for j in range(0, len(extras), max_waits):
                    nop_inst = make_nop(inst.engine)
                    nop_inst.sync_info = _br.SyncInfo(
                        on_wait=extras[j:j + max_waits], on_update=[]
                    )
                    new_nops.append(nop_inst)
                si.on_wait = waits[len(waits) - max_waits:]
                insts[i:i] = new_nops
                i += len(new_nops)
            i += 1


B, D, G, P = 4, 768, 64, 256
H, HD = 12, 64
SCALE = HD ** -0.5
N_CORES = 8
GC = G // N_CORES  # groups per core
DC = D // 128      # 128-partition chunks of the channel dim
F32 = mybir.dt.float32
F32R = mybir.dt.float32r
BF16 = mybir.dt.bfloat16

AF = mybir.ActivationFunctionType


def build_nc():
    nc = bass.Bass("TRN2")
    x_d = nc.dram_tensor("x", [B, D, GC, P], F32R, kind="ExternalInput")
    mt_d = nc.dram_tensor("maskT", [GC, P, P], BF16, kind="ExternalInput")
    wq_d = nc.dram_tensor("wqkvT", [D, 3 * D], F32R, kind="ExternalInput")
    wp_d = nc.dram_tensor("wprojT", [D, D], F32R, kind="ExternalInput")
    bp_d = nc.dram_tensor("bproj", [D], F32, kind="ExternalInput")
    on_d = nc.dram_tensor("ones_c", [128, 64], BF16, kind="ExternalInput")
    pc_d = nc.dram_tensor("pair_c", [97, 128], F32R, kind="ExternalInput")
    rz_d = nc.dram_tensor("rones", [97, P], F32R, kind="ExternalInput")
    o_d = nc.dram_tensor("out", [B, D, GC, P], F32, kind="ExternalOutput")

    with tile.TileContext(nc) as tc, nc.allow_low_precision(
        reason="float32r/bf16 matmul inputs; accumulation stays fp32"
    ):
        with (
            tc.tile_pool(name="wpool", bufs=1) as wpool,
            tc.tile_pool(name="maskp", bufs=2) as maskp,
            tc.tile_pool(name="xp", bufs=2) as xp,
            tc.tile_pool(name="qkvp", bufs=1) as qkvp,
            tc.tile_pool(name="ptp", bufs=6) as ptp,
            tc.tile_pool(name="attp", bufs=1) as attp,
            tc.tile_pool(name="pjp", bufs=1) as pjp,
            tc.tile_pool(name="otp", bufs=2) as otp,
            tc.tile_pool(name="psa", bufs=5, space="PSUM") as psa,
            tc.tile_pool(name="psb", bufs=3, space="PSUM") as psb,
        ):
            wq = wpool.tile([128, DC, 3 * D], F32R)
            for dc in range(DC):
                nc.sync.dma_start(out=wq[:, dc, :], in_=wq_d[128 * dc:128 * (dc + 1), :])
            wp = wpool.tile([128, DC, D], F32R)
            for dc in range(DC):
                nc.sync.dma_start(out=wp[:, dc, :], in_=wp_d[128 * dc:128 * (dc + 1), :])
            bias = wpool.tile([128, DC], F32)
            nc.sync.dma_start(out=bias, in_=bp_d[:].rearrange("(c p) -> p c", p=128))
            pairc = wpool.tile([97, 128], F32R)
            nc.sync.dma_start(out=pairc, in_=pc_d[:, :])
            # rowsum gather tiles: 4 heads' denominators per tile at
            # partitions {0,32,64,96} (engine writes must be 32-aligned);
            # in-between rows are kept at 1.0 (ones-init, and 1/1 stays 1
            # through the in-place reciprocal) so the zero rows of pair_c
            # never meet Inf/NaN.
            gts = []
            for ri in range(6):
                gt = wpool.tile([97, P], F32R, name=f"gt{ri}", tag=f"gt{ri}")
                nc.sync.dma_start(out=gt, in_=rz_d[:, :])
                gts.append(gt)
            gt_i = 0
            vt = wpool.tile([128, 2, 2, H, HD + 1], BF16)
            nc.sync.dma_start(
                out=vt[:, :, :, :, HD],
                in_=on_d[:, 0:48].rearrange("p (a b h) -> p a b h", a=2, b=2),
            )

            for gi in range(GC):
                mk = maskp.tile([128, 2, P], BF16, tag="mk")
                for kc in range(2):
                    nc.sync.dma_start(
                        out=mk[:, kc, :], in_=mt_d[gi, 128 * kc:128 * (kc + 1), :]
                    )
                for bp2 in range(2):  # batch pairs
                    xt = xp.tile([128, DC, 2 * P], F32R, tag="xt")
                    for b2 in range(2):
                        b = 2 * bp2 + b2
                        for dc in range(DC):
                            nc.sync.dma_start(
                                out=xt[:, dc, P * b2:P * (b2 + 1)],
                                in_=x_d[b, 128 * dc:128 * (dc + 1), gi, :],
                            )
                    qt = qkvp.tile([128, DC, 2 * P], F32R, tag="qt")
                    kt = qkvp.tile([128, DC, 2 * P], F32R, tag="kt")
                    # q,k projection: [e, token], both batches as N=512 moving
                    for c in range(2 * DC):
                        ps = psa.tile([128, 2 * P], F32, tag="psa")
                        for dc in range(DC):
                            nc.tensor.matmul(
                                ps,
                                wq[:, dc, 128 * c:128 * (c + 1)],
                                xt[:, dc, :],
                                start=(dc == 0),
                                stop=(dc == DC - 1),
                            )
                        dst = qt if c < DC else kt
                        nc.vector.tensor_copy(dst[:, c % DC, :], ps)
                    # v projection, transposed: [token, e] (x chunks stationary)
                    for b2 in range(2):
                        for tkc in range(2):
                            for nh in range(2):
                                psv = psa.tile([128, 384], F32, tag="psa")
                                t0 = P * b2 + 128 * tkc
                                for dc in range(DC):
                                    nc.tensor.matmul(
                                        psv,
                                        xt[:, dc, t0:t0 + 128],
                                        wq[:, dc, 2 * D + 384 * nh:2 * D + 384 * (nh + 1)],
                                        start=(dc == 0),
                                        stop=(dc == DC - 1),
                                    )
                                nc.scalar.copy(
                                    vt[:, b2, tkc, 6 * nh:6 * (nh + 1), 0:HD],
                                    psv[:, :].rearrange("p (h d) -> p h d", h=6),
                                )

                    pjt = pjp.tile([128, DC, 2 * P], F32R, tag="pjt")
                    for b2 in range(2):
                        at = attp.tile([128, DC, P], F32, tag="at")
                        grp_tiles = [gts[(gt_i + j) % 6] for j in range(3)]
                        gt_i += 3
                        ops2 = None
                        for h in range(H):
                            c, half = divmod(h, 2)
                            r0 = 64 * half
                            gt = grp_tiles[h // 4]
                            grow = 32 * (h % 4)
                            sps = psa.tile([128, 2 * P], F32, tag="psa")
                            for kc in range(2):
                                nc.tensor.matmul(
                                    sps[:, P * kc:P * (kc + 1)],
                                    kt[r0:r0 + 64, c, P * b2 + 128 * kc:P * b2 + 128 * (kc + 1)],
                                    qt[r0:r0 + 64, c, P * b2:P * (b2 + 1)],
                                    start=True,
                                    stop=True,
                                )
                            pt = ptp.tile([128, 2 * P], BF16, tag="pt")
                            nc.scalar.activation(pt, sps, AF.Exp, scale=SCALE)
                            nc.gpsimd.tensor_mul(
                                pt, pt, mk[:, :, :].rearrange("p a q -> p (a q)")
                            )
                            if half == 0:
                                ops2 = psb.tile([HD + 1, 2 * P], F32, tag="psb")
                            ops = ops2[:, P * half:P * (half + 1)]
                            for kc in range(2):
                                nc.tensor.matmul(
                                    ops,
                                    vt[:, b2, kc, h, :],
                                    pt[:, P * kc:P * (kc + 1)],
                                    start=(kc == 0),
                                    stop=(kc == 1),
                                )
                            nc.scalar.copy(gt[grow:grow + 1, :], ops[HD:HD + 1, :])
                            nc.vector.tensor_copy(at[r0:r0 + 64, c, :], ops[0:HD, :])
                        # batched softmax denominators + partition broadcast,
                        # off the per-head critical path
                        for grp in range(3):
                            gt = grp_tiles[grp]
                            nc.vector.reciprocal(gt[:, :], gt[:, :])
                            for pr in range(2):
                                Rps = psb.tile([128, P], F32, tag="psb")
                                nc.tensor.matmul(
                                    Rps,
                                    pairc[64 * pr:64 * pr + 33, :],
                                    gt[64 * pr:64 * pr + 33, :],
                                    start=True,
                                    stop=True,
                                )
                                for half in range(2):
                                    h = 4 * grp + 2 * pr + half
                                    c = h // 2
                                    r0 = 64 * half
                                    nc.vector.tensor_mul(
                                        pjt[r0:r0 + 64, c, P * b2:P * (b2 + 1)],
                                        at[r0:r0 + 64, c, :],
                                        Rps[r0:r0 + 64, :],
                                    )
                    # output projection, both batches as N=512 moving
                    for ec in range(DC):
                        pps = psa.tile([128, 2 * P], F32, tag="psa")
                        for dc in range(DC):
                            nc.tensor.matmul(
                                pps,
                                wp[:, dc, 128 * ec:128 * (ec + 1)],
                                pjt[:, dc, :],
                                start=(dc == 0),
                                stop=(dc == DC - 1),
                            )
                        ot = otp.tile([128, 2 * P], F32, tag="ot")
                        nc.scalar.activation(
                            ot, pps, AF.Identity, bias=bias[:, ec:ec + 1]
                        )
                        for b2 in range(2):
                            nc.sync.dma_start(
                                out=o_d[2 * bp2 + b2, 128 * ec:128 * (ec + 1), gi, :],
                                in_=ot[:, P * b2:P * (b2 + 1)],
                            )
    _split_excess_waits(nc)
    return nc


def _prep_in_maps(x, mask, w_qkv, w_proj, b_proj):
    x = np.ascontiguousarray(x, dtype=np.float32)
    maskT = np.ascontiguousarray(
        np.transpose(mask, (0, 2, 1)).astype(ml_dtypes.bfloat16)
    )  # [g, key, query], exact 0/1 in bf16
    wqkvT = np.ascontiguousarray(w_qkv.T.astype(np.float32))
    wprojT = np.ascontiguousarray(w_proj.T.astype(np.float32))
    b_proj = np.ascontiguousarray(b_proj, dtype=np.float32)
    pair_c = np.zeros((97, 128), dtype=np.float32)
    pair_c[0, 0:64] = 1.0
    pair_c[32, 64:128] = 1.0
    pair_c[64, 0:64] = 1.0
    pair_c[96, 64:128] = 1.0
    in_maps = []
    for i in range(N_CORES):
        gs = slice(i * GC, (i + 1) * GC)
        in_maps.append(
            {
                "x": np.ascontiguousarray(x[:, :, gs, :]),
                "maskT": np.ascontiguousarray(maskT[gs]),
                "wqkvT": wqkvT,
                "wprojT": wprojT,
                "bproj": b_proj,
                "ones_c": np.ones((128, 64), dtype=ml_dtypes.bfloat16),
                "pair_c": pair_c,
                "rones": np.ones((97, P), dtype=np.float32),
            }
        )
    return in_maps


def _run(inputs, trace=False):
    nc = build_nc()
    in_maps = _prep_in_maps(
        inputs["x"], inputs["mask"], inputs["w_qkv"], inputs["w_proj"], inputs["b_proj"]
    )
    res = run_bass_kernel_spmd(nc, in_maps, list(range(N_CORES)), trace=trace)
    out = np.concatenate([res.results[i]["out"] for i in range(N_CORES)], axis=2)
    return out.astype(np.float32), res


def kernel(x, mask, w_qkv, w_proj, b_proj):
    out, _ = _run(
        {"x": x, "mask": mask, "w_qkv": w_qkv, "w_proj": w_proj, "b_proj": b_proj}
    )
    return out


# revision 22
# speedup vs baseline: 1.3910x; 1.0189x over previous
"""Trainium2 Bass kernel for nn_AttentionPatch (patch attention block).

Reference computation (per batch b, group g):
    qkv  = w_qkv @ x[b,:,g,:]            # [2304, 256] channel matmul
    q,k,v per head (12 heads, hd=64)
    S    = (q^T k) * hd**-0.5            # [256 query, 256 key]
    P    = exp(S) * mask[g]              # masked softmax numerator
    att  = (P @ v) / rowsum(P)
    out  = w_proj @ att + b_proj

Sharding: data-parallel over the 64 groups (8 per core), zero communication.

Layout strategy (all matmuls keep channels on partitions, tokens on free):
    x_sbuf  [128d x 6, 512]  two batches side by side (N=512 moving operand)
    q,k     [hd, token] from QKV matmul;  v produced directly as [token, hd]
            (by swapping stationary/moving) so attention needs no transposes.
    S^T     [key, query] via lhsT=k_slice, rhs=q  (contraction over hd=64)
    P^T     = exp(S^T) * maskT in bf16 (ACT+DVE); mask transposed on host.
    AV      lhsT=[v|ones] [key,65] bf16, rhs=P^T -> [hd|rowsum, query]; the
            ones column yields the softmax denominator for free.
    norm    rowsums of 4 heads gathered at partitions {0,32,64,96} of one
            tile, one batched reciprocal, then per-head-pair rank-2 matmuls
            broadcast the recips across partitions (deferred to the end of
            each batch so the PE never stalls behind a reciprocal).
    proj    lhsT=w_projT chunks, rhs=normalized att [d, token].

QKV / S / proj matmuls run as float32r (full PE rate at moving dim >= 256,
~1e-4 relative error); the attention-probability path runs in bf16.
"""

import numpy as np
import ml_dtypes

import concourse.bass as bass
import concourse.tile as tile
from concourse import mybir
from concourse.bass_utils import run_bass_kernel_spmd

# The walrus build in this container rejects instructions carrying more
# sem waits than the ISA sync field supports ("Too many sync wait
# commands" in setupSyncWait). TileContext's kernel-tail drain accumulates
# one wait per live semaphore on a single SP drain, which trips that
# limit for any nontrivial kernel. Split across single-wait SP nops.
def _patched_drain_and_barrier(self, tick_clock, wait_clock):
    probe = self.nc.sync.nop(nofuse=True, hint="tile_tail_waits")
    wait_clock.add_sem_waits(
        probe.ins, tile.ScopedClock({None: tick_clock.global_clock})
    )
    waits = list(probe.ins.sync_info.on_wait or [])
    probe.ins.sync_info.on_wait = waits[:1]
    import bass_rust as _br

    for w in waits[1:]:
        ni = self.nc.sync.nop(nofuse=True, hint="tile_tail_waits")
        ni.ins.sync_info = _br.SyncInfo(on_wait=[w], on_update=[])
    self.nc.sync.drain()

    self.nc.all_engine_barrier()
    assert self.sems is not None
    popped = self.nc._tile_sem_poison_stack.pop()
    assert popped is self._sem_poison
    self.nc.clear_and_free_semaphores(list(self.sems.allocated().values()))
    self.nc.all_engine_barrier()


tile.TileContext._drain_and_barrier = _patched_drain_and_barrier


# Same walrus limit, applied generally: move excess waits onto same-engine
# nops placed immediately before the instruction (identical semantics: the
# engine blocks on the nop's waits first).
def _split_excess_waits(nc, max_waits=1):
    import bass_rust as _br

    def make_nop(engine):
        ins = nc.engines[engine].nop(hint="wait_split", nofuse=True).ins
        for bb in nc.m.functions[0].blocks:
            lst = bb.instructions
            if lst and lst[-1] is ins:
                lst.pop()
        return ins

    for bb in nc.m.functions[0].blocks:
        insts = bb.instructions
        i = 0
        while i < len(insts):
            inst = insts[i]
            si = inst.sync_info
            waits = list(si.on_wait) if si and si.on_wait else []
            if len(waits) > max_waits:
                extras = waits[: len(waits) - max_waits]
                new_nops = []
                for j in range(0, len(extras), max_waits):
                    nop_inst = make_nop(inst.engine)
                    nop_inst.sync_info = _br.SyncInfo(
                        on_wait=extras[j:j + max_waits], on_update=[]
                    )
                    new_nops.append(nop_inst)
                si.on_wait = waits[len(waits) - max_waits:]
                insts[i:i] = new_nops
                i += len(new_nops)
            i += 1


B, D, G, P = 4, 768, 64, 256
H, HD = 12, 64
SCALE = HD ** -0.5
N_CORES = 8
GC = G // N_CORES  # groups per core
DC = D // 128      # 128-partition chunks of the channel dim
F32 = mybir.dt.float32
F32R = mybir.dt.float32r
BF16 = mybir.dt.bfloat16

AF = mybir.ActivationFunctionType


def build_nc():
    nc = bass.Bass("TRN2")
    x_d = nc.dram_tensor("x", [B, D, GC, P], F32R, kind="ExternalInput")
    mt_d = nc.dram_tensor("maskT", [GC, P, P], BF16, kind="ExternalInput")
    wq_d = nc.dram_tensor("wqkvT", [D, 3 * D], F32R, kind="ExternalInput")
    wp_d = nc.dram_tensor("wprojT", [D, D], F32R, kind="ExternalInput")
    bp_d = nc.dram_tensor("bproj", [D], F32, kind="ExternalInput")
    on_d = nc.dram_tensor("ones_c", [128, 64], BF16, kind="ExternalInput")
    pc_d = nc.dram_tensor("pair_c", [97, 128], F32R, kind="ExternalInput")
    rz_d = nc.dram_tensor("rones", [97, P], F32R, kind="ExternalInput")
    o_d = nc.dram_tensor("out", [B, D, GC, P], F32, kind="ExternalOutput")

    with tile.TileContext(nc) as tc, nc.allow_low_precision(
        reason="float32r/bf16 matmul inputs; accumulation stays fp32"
    ):
        with (
            tc.tile_pool(name="wpool", bufs=1) as wpool,
            tc.tile_pool(name="maskp", bufs=2) as maskp,
            tc.tile_pool(name="xp", bufs=2) as xp,
            tc.tile_pool(name="qkvp", bufs=1) as qkvp,
            tc.tile_pool(name="ptp", bufs=6) as ptp,
            tc.tile_pool(name="attp", bufs=1) as attp,
            tc.tile_pool(name="pjp", bufs=2) as pjp,
            tc.tile_pool(name="otp", bufs=2) as otp,
            tc.tile_pool(name="psa", bufs=5, space="PSUM") as psa,
            tc.tile_pool(name="psb", bufs=3, space="PSUM") as psb,
        ):
            wq = wpool.tile([128, DC, 3 * D], F32R)
            for dc in range(DC):
                nc.sync.dma_start(out=wq[:, dc, :], in_=wq_d[128 * dc:128 * (dc + 1), :])
            wp = wpool.tile([128, DC, D], F32R)
            for dc in range(DC):
                nc.sync.dma_start(out=wp[:, dc, :], in_=wp_d[128 * dc:128 * (dc + 1), :])
            bias = wpool.tile([128, DC], F32)
            nc.sync.dma_start(out=bias, in_=bp_d[:].rearrange("(c p) -> p c", p=128))
            pairc = wpool.tile([97, 128], F32R)
            nc.sync.dma_start(out=pairc, in_=pc_d[:, :])
            # rowsum gather tiles: 4 heads' denominators per tile at
            # partitions {0,32,64,96} (engine writes must be 32-aligned);
            # in-between rows are kept at 1.0 (ones-init, and 1/1 stays 1
            # through the in-place reciprocal) so the zero rows of pair_c
            # never meet Inf/NaN.
            gts = []
            for ri in range(6):
                gt = wpool.tile([97, P], F32R, name=f"gt{ri}", tag=f"gt{ri}")
                nc.sync.dma_start(out=gt, in_=rz_d[:, :])
                gts.append(gt)
            gt_i = 0
            vt = wpool.tile([128, 2, 2, H, HD + 1], BF16)
            nc.sync.dma_start(
                out=vt[:, :, :, :, HD],
                in_=on_d[:, 0:48].rearrange("p (a b h) -> p a b h", a=2, b=2),
            )

            def emit_proj_chunk(prev, ec):
                # one output-projection chunk of the PREVIOUS iteration,
                # woven into the current attention phase as PE filler
                pjt_p, gi_p, bp2_p = prev
                pps = psa.tile([128, 2 * P], F32, tag="psa", name=f"pps{gi_p}_{bp2_p}_{ec}")
                for dc in range(DC):
                    nc.tensor.matmul(
                        pps,
                        wp[:, dc, 128 * ec:128 * (ec + 1)],
                        pjt_p[:, dc, :],
                        start=(dc == 0),
                        stop=(dc == DC - 1),
                    )
                ot = otp.tile([128, 2 * P], F32, tag="ot", name=f"ot{gi_p}_{bp2_p}_{ec}")
                nc.scalar.activation(ot, pps, AF.Identity, bias=bias[:, ec:ec + 1])
                for b2 in range(2):
                    nc.sync.dma_start(
                        out=o_d[2 * bp2_p + b2, 128 * ec:128 * (ec + 1), gi_p, :],
                        in_=ot[:, P * b2:P * (b2 + 1)],
                    )

            prev = None  # (pjt, gi, bp2) awaiting output projection
            for gi in range(GC):
                mk = maskp.tile([128, 2, P], BF16, tag="mk")
                for kc in range(2):
                    nc.sync.dma_start(
                        out=mk[:, kc, :], in_=mt_d[gi, 128 * kc:128 * (kc + 1), :]
                    )
                for bp2 in range(2):  # batch pairs
                    xt = xp.tile([128, DC, 2 * P], F32R, tag="xt")
                    for b2 in range(2):
                        b = 2 * bp2 + b2
                        for dc in range(DC):
                            nc.sync.dma_start(
                                out=xt[:, dc, P * b2:P * (b2 + 1)],
                                in_=x_d[b, 128 * dc:128 * (dc + 1), gi, :],
                            )
                    qt = qkvp.tile([128, DC, 2 * P], F32R, tag="qt")
                    kt = qkvp.tile([128, DC, 2 * P], F32R, tag="kt")
                    # q,k projection: [e, token], both batches as N=512 moving
                    for c in range(2 * DC):
                        ps = psa.tile([128, 2 * P], F32, tag="psa")
                        for dc in range(DC):
                            nc.tensor.matmul(
                                ps,
                                wq[:, dc, 128 * c:128 * (c + 1)],
                                xt[:, dc, :],
                                start=(dc == 0),
                                stop=(dc == DC - 1),
                            )
                        dst = qt if c < DC else kt
                        nc.vector.tensor_copy(dst[:, c % DC, :], ps)
                    # v projection, transposed: [token, e] (x chunks stationary)
                    for b2 in range(2):
                        for tkc in range(2):
                            for nh in range(2):
                                psv = psa.tile([128, 384], F32, tag="psa")
                                t0 = P * b2 + 128 * tkc
                                for dc in range(DC):
                                    nc.tensor.matmul(
                                        psv,
                                        xt[:, dc, t0:t0 + 128],
                                        wq[:, dc, 2 * D + 384 * nh:2 * D + 384 * (nh + 1)],
                                        start=(dc == 0),
                                        stop=(dc == DC - 1),
                                    )
                                nc.scalar.copy(
                                    vt[:, b2, tkc, 6 * nh:6 * (nh + 1), 0:HD],
                                    psv[:, :].rearrange("p (h d) -> p h d", h=6),
                                )

                    # attention: the two batches' independent streams are
                    # interleaved head-by-head, with the previous iteration's
                    # output projection woven in, so the PE never sits in a
                    # low-duty phase (which would re-throttle its clock)
                    pjt = pjp.tile([128, DC, 2 * P], F32R, tag="pjt")
                    ats = [
                        attp.tile([128, DC, P], F32, tag=f"at{b2}", name=f"at{b2}")
                        for b2 in range(2)
                    ]
                    grp2 = [[gts[(gt_i + 3 * b2 + j) % 6] for j in range(3)] for b2 in range(2)]
                    gt_i += 6
                    ops2 = [None, None]
                    for h in range(H):
                        c, half = divmod(h, 2)
                        r0 = 64 * half
                        grow = 32 * (h % 4)
                        for b2 in range(2):
                            at = ats[b2]
                            gt = grp2[b2][h // 4]
                            sps = psa.tile([128, 2 * P], F32, tag="psa")
                            for kc in range(2):
                                nc.tensor.matmul(
                                    sps[:, P * kc:P * (kc + 1)],
                                    kt[r0:r0 + 64, c, P * b2 + 128 * kc:P * b2 + 128 * (kc + 1)],
                                    qt[r0:r0 + 64, c, P * b2:P * (b2 + 1)],
                                    start=True,
                                    stop=True,
                                )
                            pt = ptp.tile([128, 2 * P], BF16, tag="pt")
                            nc.scalar.activation(pt, sps, AF.Exp, scale=SCALE)
                            nc.gpsimd.tensor_mul(
                                pt, pt, mk[:, :, :].rearrange("p a q -> p (a q)")
                            )
                            if half == 0:
                                ops2[b2] = psb.tile([HD + 1, 2 * P], F32, tag="psb", name=f"ops2_{b2}")
                            ops = ops2[b2][:, P * half:P * (half + 1)]
                            for kc in range(2):
                                nc.tensor.matmul(
                                    ops,
                                    vt[:, b2, kc, h, :],
                                    pt[:, P * kc:P * (kc + 1)],
                                    start=(kc == 0),
                                    stop=(kc == 1),
                                )
                            nc.scalar.copy(gt[grow:grow + 1, :], ops[HD:HD + 1, :])
                            nc.vector.tensor_copy(at[r0:r0 + 64, c, :], ops[0:HD, :])
                        if prev is not None and half == 1:
                            emit_proj_chunk(prev, h // 2)
                    prev = (pjt, gi, bp2)
                    # batched softmax denominators + partition broadcast,
                    # off the per-head critical path
                    for b2 in range(2):
                        for grp in range(3):
                            gt = grp2[b2][grp]
                            nc.vector.reciprocal(gt[:, :], gt[:, :])
                            for pr in range(2):
                                Rps = psb.tile([128, P], F32, tag="psb")
                                nc.tensor.matmul(
                                    Rps,
                                    pairc[64 * pr:64 * pr + 33, :],
                                    gt[64 * pr:64 * pr + 33, :],
                                    start=True,
                                    stop=True,
                                )
                                for half in range(2):
                                    h = 4 * grp + 2 * pr + half
                                    c = h // 2
                                    r0 = 64 * half
                                    nc.vector.tensor_mul(
                                        pjt[r0:r0 + 64, c, P * b2:P * (b2 + 1)],
                                        ats[b2][r0:r0 + 64, c, :],
                                        Rps[r0:r0 + 64, :],
                                    )
            # output projection of the final iteration
            for ec in range(DC):
                emit_proj_chunk(prev, ec)
    _split_excess_waits(nc)
    return nc


def _prep_in_maps(x, mask, w_qkv, w_proj, b_proj):
    x = np.ascontiguousarray(x, dtype=np.float32)
    maskT = np.ascontiguousarray(
        np.transpose(mask, (0, 2, 1)).astype(ml_dtypes.bfloat16)
    )  # [g, key, query], exact 0/1 in bf16
    wqkvT = np.ascontiguousarray(w_qkv.T.astype(np.float32))
    wprojT = np.ascontiguousarray(w_proj.T.astype(np.float32))
    b_proj = np.ascontiguousarray(b_proj, dtype=np.float32)
    pair_c = np.zeros((97, 128), dtype=np.float32)
    pair_c[0, 0:64] = 1.0
    pair_c[32, 64:128] = 1.0
    pair_c[64, 0:64] = 1.0
    pair_c[96, 64:128] = 1.0
    in_maps = []
    for i in range(N_CORES):
        gs = slice(i * GC, (i + 1) * GC)
        in_maps.append(
            {
                "x": np.ascontiguousarray(x[:, :, gs, :]),
                "maskT": np.ascontiguousarray(maskT[gs]),
                "wqkvT": wqkvT,
                "wprojT": wprojT,
                "bproj": b_proj,
                "ones_c": np.ones((128, 64), dtype=ml_dtypes.bfloat16),
                "pair_c": pair_c,
                "rones": np.ones((97, P), dtype=np.float32),
            }
        )
    return in_maps


def _run(inputs, trace=False):
    nc = build_nc()
    in_maps = _prep_in_maps(
        inputs["x"], inputs["mask"], inputs["w_qkv"], inputs["w_proj"], inputs["b_proj"]
    )
    res = run_bass_kernel_spmd(nc, in_maps, list(range(N_CORES)), trace=trace)
    out = np.concatenate([res.results[i]["out"] for i in range(N_CORES)], axis=2)
    return out.astype(np.float32), res


def kernel(x, mask, w_qkv, w_proj, b_proj):
    out, _ = _run(
        {"x": x, "mask": mask, "w_qkv": w_qkv, "w_proj": w_proj, "b_proj": b_proj}
    )
    return out


# revision 23
# speedup vs baseline: 1.4592x; 1.0490x over previous
"""Trainium2 Bass kernel for nn_AttentionPatch (patch attention block).

Reference computation (per batch b, group g):
    qkv  = w_qkv @ x[b,:,g,:]            # [2304, 256] channel matmul
    q,k,v per head (12 heads, hd=64)
    S    = (q^T k) * hd**-0.5            # [256 query, 256 key]
    P    = exp(S) * mask[g]              # masked softmax numerator
    att  = (P @ v) / rowsum(P)
    out  = w_proj @ att + b_proj

Sharding: data-parallel over the 64 groups (8 per core), zero communication.

Layout strategy (all matmuls keep channels on partitions, tokens on free):
    x_sbuf  [128d x 6, 512]  two batches side by side (N=512 moving operand)
    q,k     [hd, token] from QKV matmul;  v produced directly as [token, hd]
            (by swapping stationary/moving) so attention needs no transposes.
    S^T     [key, query] via lhsT=k_slice, rhs=q  (contraction over hd=64)
    P^T     = exp(S^T) * maskT in bf16 (ACT+DVE); mask transposed on host.
    AV      lhsT=[v|ones] [key,65] bf16, rhs=P^T -> [hd|rowsum, query]; the
            ones column yields the softmax denominator for free.
    norm    rowsums of 4 heads gathered at partitions {0,32,64,96} of one
            tile, one batched reciprocal, then per-head-pair rank-2 matmuls
            broadcast the recips across partitions (deferred to the end of
            each batch so the PE never stalls behind a reciprocal).
    proj    lhsT=w_projT chunks, rhs=normalized att [d, token].

QKV / S / proj matmuls run as float32r (full PE rate at moving dim >= 256,
~1e-4 relative error); the attention-probability path runs in bf16.
"""

import numpy as np
import ml_dtypes

import concourse.bass as bass
import concourse.tile as tile
from concourse import mybir
from concourse.bass_utils import run_bass_kernel_spmd

# The walrus build in this container rejects instructions carrying more
# sem waits than the ISA sync field supports ("Too many sync wait
# commands" in setupSyncWait). TileContext's kernel-tail drain accumulates
# one wait per live semaphore on a single SP drain, which trips that
# limit for any nontrivial kernel. Split across single-wait SP nops.
def _patched_drain_and_barrier(self, tick_clock, wait_clock):
    probe = self.nc.sync.nop(nofuse=True, hint="tile_tail_waits")
    wait_clock.add_sem_waits(
        probe.ins, tile.ScopedClock({None: tick_clock.global_clock})
    )
    waits = list(probe.ins.sync_info.on_wait or [])
    probe.ins.sync_info.on_wait = waits[:1]
    import bass_rust as _br

    for w in waits[1:]:
        ni = self.nc.sync.nop(nofuse=True, hint="tile_tail_waits")
        ni.ins.sync_info = _br.SyncInfo(on_wait=[w], on_update=[])
    self.nc.sync.drain()

    self.nc.all_engine_barrier()
    assert self.sems is not None
    popped = self.nc._tile_sem_poison_stack.pop()
    assert popped is self._sem_poison
    self.nc.clear_and_free_semaphores(list(self.sems.allocated().values()))
    self.nc.all_engine_barrier()


tile.TileContext._drain_and_barrier = _patched_drain_and_barrier


# Same walrus limit, applied generally: move excess waits onto same-engine
# nops placed immediately before the instruction (identical semantics: the
# engine blocks on the nop's waits first).
def _split_excess_waits(nc, max_waits=1):
    import bass_rust as _br

    def make_nop(engine):
        ins = nc.engines[engine].nop(hint="wait_split", nofuse=True).ins
        for bb in nc.m.functions[0].blocks:
            lst = bb.instructions
            if lst and lst[-1] is ins:
                lst.pop()
        return ins

    for bb in nc.m.functions[0].blocks:
        insts = bb.instructions
        i = 0
        while i < len(insts):
            inst = insts[i]
            si = inst.sync_info
            waits = list(si.on_wait) if si and si.on_wait else []
            if len(waits) > max_waits:
                extras = waits[: len(waits) - max_waits]
                new_nops = []
                for j in range(0, len(extras), max_waits):
                    nop_inst = make_nop(inst.engine)
                    nop_inst.sync_info = _br.SyncInfo(
                        on_wait=extras[j:j + max_waits], on_update=[]
                    )
                    new_nops.append(nop_inst)
                si.on_wait = waits[len(waits) - max_waits:]
                insts[i:i] = new_nops
                i += len(new_nops)
            i += 1


B, D, G, P = 4, 768, 64, 256
H, HD = 12, 64
SCALE = HD ** -0.5
N_CORES = 8
GC = G // N_CORES  # groups per core
DC = D // 128      # 128-partition chunks of the channel dim
F32 = mybir.dt.float32
F32R = mybir.dt.float32r
BF16 = mybir.dt.bfloat16

AF = mybir.ActivationFunctionType


def build_nc():
    nc = bass.Bass("TRN2")
    x_d = nc.dram_tensor("x", [B, D, GC, P], BF16, kind="ExternalInput")
    mt_d = nc.dram_tensor("maskT", [GC, P, P], BF16, kind="ExternalInput")
    wq_d = nc.dram_tensor("wqkvT", [D, 3 * D], BF16, kind="ExternalInput")
    wp_d = nc.dram_tensor("wprojT", [D, D], BF16, kind="ExternalInput")
    bp_d = nc.dram_tensor("bproj", [D], F32, kind="ExternalInput")
    on_d = nc.dram_tensor("ones_c", [128, 64], BF16, kind="ExternalInput")
    pc_d = nc.dram_tensor("pair_c", [97, 128], F32R, kind="ExternalInput")
    rz_d = nc.dram_tensor("rones", [97, P], F32R, kind="ExternalInput")
    o_d = nc.dram_tensor("out", [B, D, GC, P], F32, kind="ExternalOutput")

    with tile.TileContext(nc) as tc, nc.allow_low_precision(
        reason="float32r/bf16 matmul inputs; accumulation stays fp32"
    ):
        with (
            tc.tile_pool(name="wpool", bufs=1) as wpool,
            tc.tile_pool(name="maskp", bufs=2) as maskp,
            tc.tile_pool(name="xp", bufs=2) as xp,
            tc.tile_pool(name="qkvp", bufs=1) as qkvp,
            tc.tile_pool(name="ptp", bufs=6) as ptp,
            tc.tile_pool(name="attp", bufs=1) as attp,
            tc.tile_pool(name="pjp", bufs=2) as pjp,
            tc.tile_pool(name="otp", bufs=2) as otp,
            tc.tile_pool(name="psa", bufs=5, space="PSUM") as psa,
            tc.tile_pool(name="psb", bufs=3, space="PSUM") as psb,
        ):
            wq = wpool.tile([128, DC, 3 * D], BF16)
            for dc in range(DC):
                nc.sync.dma_start(out=wq[:, dc, :], in_=wq_d[128 * dc:128 * (dc + 1), :])
            wp = wpool.tile([128, DC, D], BF16)
            for dc in range(DC):
                nc.sync.dma_start(out=wp[:, dc, :], in_=wp_d[128 * dc:128 * (dc + 1), :])
            bias = wpool.tile([128, DC], F32)
            nc.sync.dma_start(out=bias, in_=bp_d[:].rearrange("(c p) -> p c", p=128))
            pairc = wpool.tile([97, 128], F32R)
            nc.sync.dma_start(out=pairc, in_=pc_d[:, :])
            # rowsum gather tiles: 4 heads' denominators per tile at
            # partitions {0,32,64,96} (engine writes must be 32-aligned);
            # in-between rows are kept at 1.0 (ones-init, and 1/1 stays 1
            # through the in-place reciprocal) so the zero rows of pair_c
            # never meet Inf/NaN.
            gts = []
            for ri in range(6):
                gt = wpool.tile([97, P], F32R, name=f"gt{ri}", tag=f"gt{ri}")
                nc.sync.dma_start(out=gt, in_=rz_d[:, :])
                gts.append(gt)
            gt_i = 0
            vt = wpool.tile([128, 2, 2, H, HD + 1], BF16)
            nc.sync.dma_start(
                out=vt[:, :, :, :, HD],
                in_=on_d[:, 0:48].rearrange("p (a b h) -> p a b h", a=2, b=2),
            )

            def emit_proj_chunk(prev, ec):
                # one output-projection chunk of the PREVIOUS iteration,
                # woven into the current attention phase as PE filler
                pjt_p, gi_p, bp2_p = prev
                pps = psa.tile([128, 2 * P], F32, tag="psa", name=f"pps{gi_p}_{bp2_p}_{ec}")
                for dc in range(DC):
                    nc.tensor.matmul(
                        pps,
                        wp[:, dc, 128 * ec:128 * (ec + 1)],
                        pjt_p[:, dc, :],
                        start=(dc == 0),
                        stop=(dc == DC - 1),
                    )
                ot = otp.tile([128, 2 * P], F32, tag="ot", name=f"ot{gi_p}_{bp2_p}_{ec}")
                nc.scalar.activation(ot, pps, AF.Identity, bias=bias[:, ec:ec + 1])
                for b2 in range(2):
                    nc.sync.dma_start(
                        out=o_d[2 * bp2_p + b2, 128 * ec:128 * (ec + 1), gi_p, :],
                        in_=ot[:, P * b2:P * (b2 + 1)],
                    )

            prev = None  # (pjt, gi, bp2) awaiting output projection
            for gi in range(GC):
                mk = maskp.tile([128, 2, P], BF16, tag="mk")
                for kc in range(2):
                    nc.sync.dma_start(
                        out=mk[:, kc, :], in_=mt_d[gi, 128 * kc:128 * (kc + 1), :]
                    )
                for bp2 in range(2):  # batch pairs
                    xt = xp.tile([128, DC, 2 * P], BF16, tag="xt")
                    for b2 in range(2):
                        b = 2 * bp2 + b2
                        for dc in range(DC):
                            nc.sync.dma_start(
                                out=xt[:, dc, P * b2:P * (b2 + 1)],
                                in_=x_d[b, 128 * dc:128 * (dc + 1), gi, :],
                            )
                    qt = qkvp.tile([128, DC, 2 * P], BF16, tag="qt")
                    kt = qkvp.tile([128, DC, 2 * P], BF16, tag="kt")
                    # q,k projection: [e, token], both batches as N=512 moving
                    for c in range(2 * DC):
                        ps = psa.tile([128, 2 * P], F32, tag="psa")
                        for dc in range(DC):
                            nc.tensor.matmul(
                                ps,
                                wq[:, dc, 128 * c:128 * (c + 1)],
                                xt[:, dc, :],
                                start=(dc == 0),
                                stop=(dc == DC - 1),
                            )
                        dst = qt if c < DC else kt
                        nc.vector.tensor_copy(dst[:, c % DC, :], ps)
                    # v projection, transposed: [token, e] (x chunks stationary)
                    for b2 in range(2):
                        for tkc in range(2):
                            for nh in range(2):
                                psv = psa.tile([128, 384], F32, tag="psa")
                                t0 = P * b2 + 128 * tkc
                                for dc in range(DC):
                                    nc.tensor.matmul(
                                        psv,
                                        xt[:, dc, t0:t0 + 128],
                                        wq[:, dc, 2 * D + 384 * nh:2 * D + 384 * (nh + 1)],
                                        start=(dc == 0),
                                        stop=(dc == DC - 1),
                                    )
                                nc.scalar.copy(
                                    vt[:, b2, tkc, 6 * nh:6 * (nh + 1), 0:HD],
                                    psv[:, :].rearrange("p (h d) -> p h d", h=6),
                                )

                    # attention: the two batches' independent streams are
                    # interleaved head-by-head, with the previous iteration's
                    # output projection woven in, so the PE never sits in a
                    # low-duty phase (which would re-throttle its clock)
                    pjt = pjp.tile([128, DC, 2 * P], BF16, tag="pjt")
                    ats = [
                        attp.tile([128, DC, P], BF16, tag=f"at{b2}", name=f"at{b2}")
                        for b2 in range(2)
                    ]
                    grp2 = [[gts[(gt_i + 3 * b2 + j) % 6] for j in range(3)] for b2 in range(2)]
                    gt_i += 6
                    ops2 = [None, None]
                    for h in range(H):
                        c, half = divmod(h, 2)
                        r0 = 64 * half
                        grow = 32 * (h % 4)
                        for b2 in range(2):
                            at = ats[b2]
                            gt = grp2[b2][h // 4]
                            sps = psa.tile([128, 2 * P], F32, tag="psa")
                            for kc in range(2):
                                nc.tensor.matmul(
                                    sps[:, P * kc:P * (kc + 1)],
                                    kt[r0:r0 + 64, c, P * b2 + 128 * kc:P * b2 + 128 * (kc + 1)],
                                    qt[r0:r0 + 64, c, P * b2:P * (b2 + 1)],
                                    start=True,
                                    stop=True,
                                )
                            pt = ptp.tile([128, 2 * P], BF16, tag="pt")
                            nc.scalar.activation(pt, sps, AF.Exp, scale=SCALE)
                            nc.gpsimd.tensor_mul(
                                pt, pt, mk[:, :, :].rearrange("p a q -> p (a q)")
                            )
                            if half == 0:
                                ops2[b2] = psb.tile([HD + 1, 2 * P], F32, tag="psb", name=f"ops2_{b2}")
                            ops = ops2[b2][:, P * half:P * (half + 1)]
                            for kc in range(2):
                                nc.tensor.matmul(
                                    ops,
                                    vt[:, b2, kc, h, :],
                                    pt[:, P * kc:P * (kc + 1)],
                                    start=(kc == 0),
                                    stop=(kc == 1),
                                )
                            nc.scalar.copy(gt[grow:grow + 1, :], ops[HD:HD + 1, :])
                            nc.vector.tensor_copy(at[r0:r0 + 64, c, :], ops[0:HD, :])
                        if prev is not None and half == 1:
                            emit_proj_chunk(prev, h // 2)
                    prev = (pjt, gi, bp2)
                    # batched softmax denominators + partition broadcast,
                    # off the per-head critical path
                    for b2 in range(2):
                        for grp in range(3):
                            gt = grp2[b2][grp]
                            nc.vector.reciprocal(gt[:, :], gt[:, :])
                            for pr in range(2):
                                Rps = psb.tile([128, P], F32, tag="psb")
                                nc.tensor.matmul(
                                    Rps,
                                    pairc[64 * pr:64 * pr + 33, :],
                                    gt[64 * pr:64 * pr + 33, :],
                                    start=True,
                                    stop=True,
                                )
                                for half in range(2):
                                    h = 4 * grp + 2 * pr + half
                                    c = h // 2
                                    r0 = 64 * half
                                    nc.vector.tensor_mul(
                                        pjt[r0:r0 + 64, c, P * b2:P * (b2 + 1)],
                                        ats[b2][r0:r0 + 64, c, :],
                                        Rps[r0:r0 + 64, :],
                                    )
            # output projection of the final iteration
            for ec in range(DC):
                emit_proj_chunk(prev, ec)
    _split_excess_waits(nc)
    return nc


def _prep_in_maps(x, mask, w_qkv, w_proj, b_proj):
    x = np.ascontiguousarray(x).astype(ml_dtypes.bfloat16)
    maskT = np.ascontiguousarray(
        np.transpose(mask, (0, 2, 1)).astype(ml_dtypes.bfloat16)
    )  # [g, key, query], exact 0/1 in bf16
    wqkvT = np.ascontiguousarray(w_qkv.T).astype(ml_dtypes.bfloat16)
    wprojT = np.ascontiguousarray(w_proj.T).astype(ml_dtypes.bfloat16)
    b_proj = np.ascontiguousarray(b_proj, dtype=np.float32)
    pair_c = np.zeros((97, 128), dtype=np.float32)
    pair_c[0, 0:64] = 1.0
    pair_c[32, 64:128] = 1.0
    pair_c[64, 0:64] = 1.0
    pair_c[96, 64:128] = 1.0
    in_maps = []
    for i in range(N_CORES):
        gs = slice(i * GC, (i + 1) * GC)
        in_maps.append(
            {
                "x": np.ascontiguousarray(x[:, :, gs, :]),
                "maskT": np.ascontiguousarray(maskT[gs]),
                "wqkvT": wqkvT,
                "wprojT": wprojT,
                "bproj": b_proj,
                "ones_c": np.ones((128, 64), dtype=ml_dtypes.bfloat16),
                "pair_c": pair_c,
                "rones": np.ones((97, P), dtype=np.float32),
            }
        )
    return in_maps


def _run(inputs, trace=False):
    nc = build_nc()
    in_maps = _prep_in_maps(
        inputs["x"], inputs["mask"], inputs["w_qkv"], inputs["w_proj"], inputs["b_proj"]
    )
    res = run_bass_kernel_spmd(nc, in_maps, list(range(N_CORES)), trace=trace)
    out = np.concatenate([res.results[i]["out"] for i in range(N_CORES)], axis=2)
    return out.astype(np.float32), res


def kernel(x, mask, w_qkv, w_proj, b_proj):
    out, _ = _run(
        {"x": x, "mask": mask, "w_qkv": w_qkv, "w_proj": w_proj, "b_proj": b_proj}
    )
    return out


# revision 26
# speedup vs baseline: 1.5751x; 1.0794x over previous
"""Trainium2 Bass kernel for nn_AttentionPatch (patch attention block).

Reference computation (per batch b, group g):
    qkv  = w_qkv @ x[b,:,g,:]            # [2304, 256] channel matmul
    q,k,v per head (12 heads, hd=64)
    S    = (q^T k) * hd**-0.5            # [256 query, 256 key]
    P    = exp(S) * mask[g]              # masked softmax numerator
    att  = (P @ v) / rowsum(P)
    out  = w_proj @ att + b_proj

Sharding: data-parallel over the 64 groups (8 per core), zero communication.

Layout strategy (all matmuls keep channels on partitions, tokens on free):
    x_sbuf  [128d x 6, 512]  two batches side by side (N=512 moving operand)
    q,k     [hd, token] from QKV matmul;  v produced directly as [token, hd]
            (by swapping stationary/moving) so attention needs no transposes.
    S^T     [key, query] via lhsT=k_slice, rhs=q  (contraction over hd=64)
    P^T     = exp(S^T) * maskT in bf16 (ACT+DVE); mask transposed on host.
    AV      lhsT=[v|ones] [key,65] bf16, rhs=P^T -> [hd|rowsum, query]; the
            ones column yields the softmax denominator for free.
    norm    rowsums of 4 heads gathered at partitions {0,32,64,96} of one
            tile, one batched reciprocal, then per-head-pair rank-2 matmuls
            broadcast the recips across partitions (deferred to the end of
            each batch so the PE never stalls behind a reciprocal).
    proj    lhsT=w_projT chunks, rhs=normalized att [d, token].

QKV / S / proj matmuls run as float32r (full PE rate at moving dim >= 256,
~1e-4 relative error); the attention-probability path runs in bf16.
"""

import numpy as np
import ml_dtypes

import concourse.bass as bass
import concourse.tile as tile
from concourse import mybir
from concourse.bass_utils import run_bass_kernel_spmd

# The walrus build in this container rejects instructions carrying more
# sem waits than the ISA sync field supports ("Too many sync wait
# commands" in setupSyncWait). TileContext's kernel-tail drain accumulates
# one wait per live semaphore on a single SP drain, which trips that
# limit for any nontrivial kernel. Split across single-wait SP nops.
def _patched_drain_and_barrier(self, tick_clock, wait_clock):
    probe = self.nc.sync.nop(nofuse=True, hint="tile_tail_waits")
    wait_clock.add_sem_waits(
        probe.ins, tile.ScopedClock({None: tick_clock.global_clock})
    )
    waits = list(probe.ins.sync_info.on_wait or [])
    probe.ins.sync_info.on_wait = waits[:1]
    import bass_rust as _br

    for w in waits[1:]:
        ni = self.nc.sync.nop(nofuse=True, hint="tile_tail_waits")
        ni.ins.sync_info = _br.SyncInfo(on_wait=[w], on_update=[])
    self.nc.sync.drain()

    self.nc.all_engine_barrier()
    assert self.sems is not None
    popped = self.nc._tile_sem_poison_stack.pop()
    assert popped is self._sem_poison
    self.nc.clear_and_free_semaphores(list(self.sems.allocated().values()))
    self.nc.all_engine_barrier()


tile.TileContext._drain_and_barrier = _patched_drain_and_barrier


# Same walrus limit, applied generally: move excess waits onto same-engine
# nops placed immediately before the instruction (identical semantics: the
# engine blocks on the nop's waits first).
def _split_excess_waits(nc, max_waits=1):
    import bass_rust as _br

    def make_nop(engine):
        ins = nc.engines[engine].nop(hint="wait_split", nofuse=True).ins
        for bb in nc.m.functions[0].blocks:
            lst = bb.instructions
            if lst and lst[-1] is ins:
                lst.pop()
        return ins

    for bb in nc.m.functions[0].blocks:
        insts = bb.instructions
        i = 0
        while i < len(insts):
            inst = insts[i]
            si = inst.sync_info
            waits = list(si.on_wait) if si and si.on_wait else []
            if len(waits) > max_waits:
                extras = waits[: len(waits) - max_waits]
                new_nops = []
                for j in range(0, len(extras), max_waits):
                    nop_inst = make_nop(inst.engine)
                    nop_inst.sync_info = _br.SyncInfo(
                        on_wait=extras[j:j + max_waits], on_update=[]
                    )
                    new_nops.append(nop_inst)
                si.on_wait = waits[len(waits) - max_waits:]
                insts[i:i] = new_nops
                i += len(new_nops)
            i += 1


B, D, G, P = 4, 768, 64, 256
H, HD = 12, 64
SCALE = HD ** -0.5
N_CORES = 8
GC = G // N_CORES  # groups per core
DC = D // 128      # 128-partition chunks of the channel dim
F32 = mybir.dt.float32
F32R = mybir.dt.float32r
BF16 = mybir.dt.bfloat16

AF = mybir.ActivationFunctionType


def build_nc():
    nc = bass.Bass("TRN2")
    x_d = nc.dram_tensor("x", [B, D, GC, P], BF16, kind="ExternalInput")
    mt_d = nc.dram_tensor("maskT", [GC, P, P], BF16, kind="ExternalInput")
    wq_d = nc.dram_tensor("wqkvT", [D, 3 * D], BF16, kind="ExternalInput")
    wp_d = nc.dram_tensor("wprojT", [D, D], BF16, kind="ExternalInput")
    bp_d = nc.dram_tensor("bproj", [D], F32, kind="ExternalInput")
    on_d = nc.dram_tensor("ones_c", [128, 64], BF16, kind="ExternalInput")
    pc_d = nc.dram_tensor("pair_c", [97, 128], F32R, kind="ExternalInput")
    rz_d = nc.dram_tensor("rones", [97, P], F32R, kind="ExternalInput")
    o_d = nc.dram_tensor("out", [B, D, GC, P], F32, kind="ExternalOutput")

    with tile.TileContext(nc) as tc, nc.allow_low_precision(
        reason="float32r/bf16 matmul inputs; accumulation stays fp32"
    ):
        with (
            tc.tile_pool(name="wpool", bufs=1) as wpool,
            tc.tile_pool(name="maskp", bufs=2) as maskp,
            tc.tile_pool(name="xp", bufs=2) as xp,
            tc.tile_pool(name="qkvp", bufs=1) as qkvp,
            tc.tile_pool(name="ptp", bufs=6) as ptp,
            tc.tile_pool(name="attp", bufs=1) as attp,
            tc.tile_pool(name="pjp", bufs=2) as pjp,
            tc.tile_pool(name="otp", bufs=2) as otp,
            tc.tile_pool(name="psa", bufs=5, space="PSUM") as psa,
            tc.tile_pool(name="psb", bufs=3, space="PSUM") as psb,
        ):
            wq = wpool.tile([128, DC, 3 * D], BF16)
            for dc in range(DC):
                nc.sync.dma_start(out=wq[:, dc, :], in_=wq_d[128 * dc:128 * (dc + 1), :])
            wp = wpool.tile([128, DC, D], BF16)
            for dc in range(DC):
                nc.sync.dma_start(out=wp[:, dc, :], in_=wp_d[128 * dc:128 * (dc + 1), :])
            bias = wpool.tile([128, DC], F32)
            nc.sync.dma_start(out=bias, in_=bp_d[:].rearrange("(c p) -> p c", p=128))
            pairc = wpool.tile([97, 128], F32R)
            nc.sync.dma_start(out=pairc, in_=pc_d[:, :])
            # rowsum gather tiles: 4 heads' denominators per tile at
            # partitions {0,32,64,96} (engine writes must be 32-aligned);
            # in-between rows are kept at 1.0 (ones-init, and 1/1 stays 1
            # through the in-place reciprocal) so the zero rows of pair_c
            # never meet Inf/NaN.
            gts = []
            for ri in range(6):
                gt = wpool.tile([97, P], F32R, name=f"gt{ri}", tag=f"gt{ri}")
                nc.sync.dma_start(out=gt, in_=rz_d[:, :])
                gts.append(gt)
            def emit_proj_chunk(prev, ec):
                # one output-projection chunk of the PREVIOUS iteration,
                # woven into the current attention phase as PE filler
                pjt_p, gi_p, bp2_p = prev
                pps = psa.tile([128, 2 * P], F32, tag="psa", name=f"pps{gi_p}_{bp2_p}_{ec}")
                for dc in range(DC):
                    nc.tensor.matmul(
                        pps,
                        wp[:, dc, 128 * ec:128 * (ec + 1)],
                        pjt_p[:, dc, :],
                        start=(dc == 0),
                        stop=(dc == DC - 1),
                    )
                ot = otp.tile([128, 2 * P], F32, tag="ot", name=f"ot{gi_p}_{bp2_p}_{ec}")
                nc.scalar.activation(ot, pps, AF.Identity, bias=bias[:, ec:ec + 1])
                for b2 in range(2):
                    nc.sync.dma_start(
                        out=o_d[2 * bp2_p + b2, 128 * ec:128 * (ec + 1), gi_p, :],
                        in_=ot[:, P * b2:P * (b2 + 1)],
                    )

            # Fully software-pipelined steady state: iteration t's QKV
            # chunk-groups are woven between iteration t-1's attention
            # heads so every engine (PE matmuls, ACT exp, DVE casts/recips,
            # GPSIMD mask) sees a uniform instruction stream — no
            # low-duty phase to re-throttle the PE clock.
            NT = GC * 2  # iterations: (gi, bp2) pairs

            def make_qkv_groups(t):
                gi, bp2 = divmod(t, 2)
                if t % 2 == 0:
                    mkt = maskp.tile([128, 2, P], BF16, tag="mk", name=f"mk{gi}")
                    for kc in range(2):
                        nc.sync.dma_start(
                            out=mkt[:, kc, :], in_=mt_d[gi, 128 * kc:128 * (kc + 1), :]
                        )
                    make_qkv_groups.mk = mkt
                mkt = make_qkv_groups.mk
                xt = xp.tile([128, DC, 2 * P], BF16, tag="xt", name=f"xt{t}")
                for b2 in range(2):
                    b = 2 * bp2 + b2
                    for dc in range(DC):
                        nc.sync.dma_start(
                            out=xt[:, dc, P * b2:P * (b2 + 1)],
                            in_=x_d[b, 128 * dc:128 * (dc + 1), gi, :],
                        )
                qt = qkvp.tile([128, DC, 2 * P], BF16, tag="qt", bufs=2, name=f"qt{t}")
                kt = qkvp.tile([128, DC, 2 * P], BF16, tag="kt", bufs=2, name=f"kt{t}")
                vt = qkvp.tile([128, 2, 2, H, HD + 1], BF16, tag="vt", bufs=2, name=f"vt{t}")
                nc.sync.dma_start(
                    out=vt[:, :, :, :, HD],
                    in_=on_d[:, 0:48].rearrange("p (a b h) -> p a b h", a=2, b=2),
                )

                groups = []

                def qk_group(c):
                    def emit():
                        ps = psa.tile([128, 2 * P], F32, tag="psa", name=f"ps{t}_{c}")
                        for dc in range(DC):
                            nc.tensor.matmul(
                                ps,
                                wq[:, dc, 128 * c:128 * (c + 1)],
                                xt[:, dc, :],
                                start=(dc == 0),
                                stop=(dc == DC - 1),
                            )
                        dst = qt if c < DC else kt
                        nc.vector.tensor_copy(dst[:, c % DC, :], ps)
                    return emit

                def v_group(b2, tkc, nh):
                    def emit():
                        psv = psa.tile([128, 384], F32, tag="psa", name=f"psv{t}_{b2}_{tkc}_{nh}")
                        t0 = P * b2 + 128 * tkc
                        for dc in range(DC):
                            nc.tensor.matmul(
                                psv,
                                xt[:, dc, t0:t0 + 128],
                                wq[:, dc, 2 * D + 384 * nh:2 * D + 384 * (nh + 1)],
                                start=(dc == 0),
                                stop=(dc == DC - 1),
                            )
                        nc.scalar.copy(
                            vt[:, b2, tkc, 6 * nh:6 * (nh + 1), 0:HD],
                            psv[:, :].rearrange("p (h d) -> p h d", h=6),
                        )
                    return emit

                for c in range(2 * DC):
                    groups.append(qk_group(c))
                for b2 in range(2):
                    for tkc in range(2):
                        for nh in range(2):
                            groups.append(v_group(b2, tkc, nh))
                return groups, (qt, kt, vt, mkt, gi, bp2)

            def make_attn_heads(ctx):
                qt, kt, vt, mkt, gi, bp2 = ctx
                pjt = pjp.tile([128, DC, 2 * P], BF16, tag="pjt", name=f"pjt{gi}_{bp2}")
                ats = [
                    attp.tile([128, DC, P], BF16, tag=f"at{b2}", name=f"at{gi}{bp2}{b2}")
                    for b2 in range(2)
                ]
                grp2 = [[gts[(3 * b2 + j)] for j in range(3)] for b2 in range(2)]
                ops2 = [None, None]
                heads = []

                def head_body(h, b2):
                    c, half = divmod(h, 2)
                    r0 = 64 * half
                    grow = 32 * (h % 4)

                    def emit():
                        at = ats[b2]
                        gt = grp2[b2][h // 4]
                        sps = psa.tile([128, 2 * P], F32, tag="psa", name=f"sps{h}_{b2}")
                        for kc in range(2):
                            nc.tensor.matmul(
                                sps[:, P * kc:P * (kc + 1)],
                                kt[r0:r0 + 64, c, P * b2 + 128 * kc:P * b2 + 128 * (kc + 1)],
                                qt[r0:r0 + 64, c, P * b2:P * (b2 + 1)],
                                start=True,
                                stop=True,
                            )
                        pt = ptp.tile([128, 2 * P], BF16, tag="pt", name=f"pt{h}_{b2}")
                        nc.scalar.activation(pt, sps, AF.Exp, scale=SCALE)
                        nc.gpsimd.tensor_mul(
                            pt, pt, mkt[:, :, :].rearrange("p a q -> p (a q)")
                        )
                        if half == 0:
                            ops2[b2] = psb.tile(
                                [HD + 1, 2 * P], F32, tag="psb", name=f"ops2_{b2}"
                            )
                        ops = ops2[b2][:, P * half:P * (half + 1)]
                        for kc in range(2):
                            nc.tensor.matmul(
                                ops,
                                vt[:, b2, kc, h, :],
                                pt[:, P * kc:P * (kc + 1)],
                                start=(kc == 0),
                                stop=(kc == 1),
                            )
                        nc.scalar.copy(gt[grow:grow + 1, :], ops[HD:HD + 1, :])
                        nc.vector.tensor_copy(at[r0:r0 + 64, c, :], ops[0:HD, :])
                    return emit

                for h in range(H):
                    for b2 in range(2):
                        heads.append(head_body(h, b2))

                def tails():
                    for b2 in range(2):
                        for grp in range(3):
                            gt = grp2[b2][grp]
                            nc.vector.reciprocal(gt[:, :], gt[:, :])
                            for pr in range(2):
                                Rps = psb.tile([128, P], F32, tag="psb", name=f"Rps{b2}_{grp}_{pr}")
                                nc.tensor.matmul(
                                    Rps,
                                    pairc[64 * pr:64 * pr + 33, :],
                                    gt[64 * pr:64 * pr + 33, :],
                                    start=True,
                                    stop=True,
                                )
                                for half in range(2):
                                    h = 4 * grp + 2 * pr + half
                                    c = h // 2
                                    r0 = 64 * half
                                    nc.vector.tensor_mul(
                                        pjt[r0:r0 + 64, c, P * b2:P * (b2 + 1)],
                                        ats[b2][r0:r0 + 64, c, :],
                                        Rps[r0:r0 + 64, :],
                                    )
                return heads, tails, (pjt, gi, bp2)

            attn = None   # (heads, tails, proj_ctx) of iteration t-1
            for t in range(NT + 1):
                groups = []
                if t < NT:
                    groups, ctx = make_qkv_groups(t)
                heads = attn[0] if attn else []
                tails = attn[1] if attn else None
                prev = attn[2] if attn else None
                # interleave: one attention head per slot, QKV groups spread
                # evenly among the slots
                n_slots = max(len(heads), 1)
                gi_q = 0
                for s in range(n_slots):
                    if heads:
                        heads[s]()
                    want = (s + 1) * len(groups) // n_slots
                    while gi_q < want:
                        groups[gi_q]()
                        gi_q += 1
                if tails is not None:
                    tails()
                if prev is not None:
                    for ec in range(DC):
                        emit_proj_chunk(prev, ec)
                attn = make_attn_heads(ctx) if t < NT else None
    _split_excess_waits(nc)
    return nc


def _prep_in_maps(x, mask, w_qkv, w_proj, b_proj):
    x = np.ascontiguousarray(x).astype(ml_dtypes.bfloat16)
    maskT = np.ascontiguousarray(
        np.transpose(mask, (0, 2, 1)).astype(ml_dtypes.bfloat16)
    )  # [g, key, query], exact 0/1 in bf16
    wqkvT = np.ascontiguousarray(w_qkv.T).astype(ml_dtypes.bfloat16)
    wprojT = np.ascontiguousarray(w_proj.T).astype(ml_dtypes.bfloat16)
    b_proj = np.ascontiguousarray(b_proj, dtype=np.float32)
    pair_c = np.zeros((97, 128), dtype=np.float32)
    pair_c[0, 0:64] = 1.0
    pair_c[32, 64:128] = 1.0
    pair_c[64, 0:64] = 1.0
    pair_c[96, 64:128] = 1.0
    in_maps = []
    for i in range(N_CORES):
        gs = slice(i * GC, (i + 1) * GC)
        in_maps.append(
            {
                "x": np.ascontiguousarray(x[:, :, gs, :]),
                "maskT": np.ascontiguousarray(maskT[gs]),
                "wqkvT": wqkvT,
                "wprojT": wprojT,
                "bproj": b_proj,
                "ones_c": np.ones((128, 64), dtype=ml_dtypes.bfloat16),
                "pair_c": pair_c,
                "rones": np.ones((97, P), dtype=np.float32),
            }
        )
    return in_maps


def _run(inputs, trace=False):
    nc = build_nc()
    in_maps = _prep_in_maps(
        inputs["x"], inputs["mask"], inputs["w_qkv"], inputs["w_proj"], inputs["b_proj"]
    )
    res = run_bass_kernel_spmd(nc, in_maps, list(range(N_CORES)), trace=trace)
    out = np.concatenate([res.results[i]["out"] for i in range(N_CORES)], axis=2)
    return out.astype(np.float32), res


def kernel(x, mask, w_qkv, w_proj, b_proj):
    out, _ = _run(
        {"x": x, "mask": mask, "w_qkv": w_qkv, "w_proj": w_proj, "b_proj": b_proj}
    )
    return out


# revision 28
# speedup vs baseline: 1.8227x; 1.1572x over previous
"""Trainium2 Bass kernel for nn_AttentionPatch (patch attention block).

Reference computation (per batch b, group g):
    qkv  = w_qkv @ x[b,:,g,:]            # [2304, 256] channel matmul
    q,k,v per head (12 heads, hd=64)
    S    = (q^T k) * hd**-0.5            # [256 query, 256 key]
    P    = exp(S) * mask[g]              # masked softmax numerator
    att  = (P @ v) / rowsum(P)
    out  = w_proj @ att + b_proj

Sharding: data-parallel over the 64 groups (8 per core), zero communication.

Layout strategy (all matmuls keep channels on partitions, tokens on free):
    x_sbuf  [128d x 6, 512]  two batches side by side (N=512 moving operand)
    q,k     [hd, token] from QKV matmul;  v produced directly as [token, hd]
            (by swapping stationary/moving) so attention needs no transposes.
    S^T     [key, query] via lhsT=k_slice, rhs=q  (contraction over hd=64)
    P^T     = exp(S^T) * maskT in bf16 (ACT+DVE); mask transposed on host.
    AV      lhsT=[v|ones] [key,65] bf16, rhs=P^T -> [hd|rowsum, query]; the
            ones column yields the softmax denominator for free.
    norm    rowsums of 4 heads gathered at partitions {0,32,64,96} of one
            tile, one batched reciprocal, then per-head-pair rank-2 matmuls
            broadcast the recips across partitions (deferred to the end of
            each batch so the PE never stalls behind a reciprocal).
    proj    lhsT=w_projT chunks, rhs=normalized att [d, token].

QKV / S / proj matmuls run as float32r (full PE rate at moving dim >= 256,
~1e-4 relative error); the attention-probability path runs in bf16.
"""

import numpy as np
import ml_dtypes

import concourse.bass as bass
import concourse.tile as tile
from concourse import mybir
from concourse.bass_utils import run_bass_kernel_spmd

# The walrus build in this container rejects instructions carrying more
# sem waits than the ISA sync field supports ("Too many sync wait
# commands" in setupSyncWait). TileContext's kernel-tail drain accumulates
# one wait per live semaphore on a single SP drain, which trips that
# limit for any nontrivial kernel. Split across single-wait SP nops.
def _patched_drain_and_barrier(self, tick_clock, wait_clock):
    probe = self.nc.sync.nop(nofuse=True, hint="tile_tail_waits")
    wait_clock.add_sem_waits(
        probe.ins, tile.ScopedClock({None: tick_clock.global_clock})
    )
    waits = list(probe.ins.sync_info.on_wait or [])
    probe.ins.sync_info.on_wait = waits[:1]
    import bass_rust as _br

    for w in waits[1:]:
        ni = self.nc.sync.nop(nofuse=True, hint="tile_tail_waits")
        ni.ins.sync_info = _br.SyncInfo(on_wait=[w], on_update=[])
    self.nc.sync.drain()

    self.nc.all_engine_barrier()
    assert self.sems is not None
    popped = self.nc._tile_sem_poison_stack.pop()
    assert popped is self._sem_poison
    self.nc.clear_and_free_semaphores(list(self.sems.allocated().values()))
    self.nc.all_engine_barrier()


tile.TileContext._drain_and_barrier = _patched_drain_and_barrier


# Same walrus limit, applied generally: move excess waits onto same-engine
# nops placed immediately before the instruction (identical semantics: the
# engine blocks on the nop's waits first).
def _split_excess_waits(nc, max_waits=1):
    import bass_rust as _br

    def make_nop(engine):
        ins = nc.engines[engine].nop(hint="wait_split", nofuse=True).ins
        for bb in nc.m.functions[0].blocks:
            lst = bb.instructions
            if lst and lst[-1] is ins:
                lst.pop()
        return ins

    for bb in nc.m.functions[0].blocks:
        insts = bb.instructions
        i = 0
        while i < len(insts):
            inst = insts[i]
            si = inst.sync_info
            waits = list(si.on_wait) if si and si.on_wait else []
            if len(waits) > max_waits:
                extras = waits[: len(waits) - max_waits]
                new_nops = []
                for j in range(0, len(extras), max_waits):
                    nop_inst = make_nop(inst.engine)
                    nop_inst.sync_info = _br.SyncInfo(
                        on_wait=extras[j:j + max_waits], on_update=[]
                    )
                    new_nops.append(nop_inst)
                si.on_wait = waits[len(waits) - max_waits:]
                insts[i:i] = new_nops
                i += len(new_nops)
            i += 1


B, D, G, P = 4, 768, 64, 256
H, HD = 12, 64
SCALE = HD ** -0.5
N_CORES = 8
GC = G // N_CORES  # groups per core
DC = D // 128      # 128-partition chunks of the channel dim
F32 = mybir.dt.float32
F32R = mybir.dt.float32r
BF16 = mybir.dt.bfloat16

AF = mybir.ActivationFunctionType


def build_nc():
    nc = bass.Bass("TRN2")
    x_d = nc.dram_tensor("x", [B, D, GC, P], BF16, kind="ExternalInput")
    mt_d = nc.dram_tensor("maskT", [GC, P, P], BF16, kind="ExternalInput")
    wq_d = nc.dram_tensor("wqkvT", [D, 3 * D], BF16, kind="ExternalInput")
    wp_d = nc.dram_tensor("wprojT", [D, D], BF16, kind="ExternalInput")
    bp_d = nc.dram_tensor("bproj", [D], F32, kind="ExternalInput")
    on_d = nc.dram_tensor("ones_c", [128, 64], BF16, kind="ExternalInput")
    pc_d = nc.dram_tensor("pair_c", [97, 128], F32R, kind="ExternalInput")
    rz_d = nc.dram_tensor("rones", [97, P], F32R, kind="ExternalInput")
    o_d = nc.dram_tensor("out", [B, D, GC, P], F32, kind="ExternalOutput")

    with tile.TileContext(nc) as tc, nc.allow_low_precision(
        reason="float32r/bf16 matmul inputs; accumulation stays fp32"
    ):
        with (
            tc.tile_pool(name="wpool", bufs=1) as wpool,
            tc.tile_pool(name="maskp", bufs=3) as maskp,
            tc.tile_pool(name="xp", bufs=2) as xp,
            tc.tile_pool(name="qkvp", bufs=1) as qkvp,
            tc.tile_pool(name="ptp", bufs=6) as ptp,
            tc.tile_pool(name="attp", bufs=2) as attp,
            tc.tile_pool(name="pjp", bufs=2) as pjp,
            tc.tile_pool(name="otp", bufs=2) as otp,
            tc.tile_pool(name="rscp", bufs=2) as rscp,
            tc.tile_pool(name="psa", bufs=5, space="PSUM") as psa,
            tc.tile_pool(name="psb", bufs=3, space="PSUM") as psb,
        ):
            wq = wpool.tile([128, DC, 3 * D], BF16)
            for dc in range(DC):
                nc.sync.dma_start(out=wq[:, dc, :], in_=wq_d[128 * dc:128 * (dc + 1), :])
            wp = wpool.tile([128, DC, D], BF16)
            for dc in range(DC):
                nc.sync.dma_start(out=wp[:, dc, :], in_=wp_d[128 * dc:128 * (dc + 1), :])
            bias = wpool.tile([128, DC], F32)
            nc.sync.dma_start(out=bias, in_=bp_d[:].rearrange("(c p) -> p c", p=128))
            pairc = wpool.tile([97, 128], F32R)
            nc.sync.dma_start(out=pairc, in_=pc_d[:, :])
            # rowsum gather tiles: 4 heads' denominators per tile at
            # partitions {0,32,64,96} (engine writes must be 32-aligned);
            # rows in between stay at their ones-init forever (they are only
            # ever read) so the fast reciprocal never sees garbage and the
            # zero rows of pair_c never meet Inf/NaN. Two sets, alternating
            # per iteration, so iteration t's gathers don't collide with
            # t-2's still-pending tail reads.
            gsets = []
            for si in range(2):
                tiles = []
                for ri in range(6):
                    gs = wpool.tile([97, P], F32, name=f"gs{si}_{ri}", tag=f"gs{si}_{ri}")
                    nc.sync.dma_start(out=gs, in_=rz_d[:, :].bitcast(F32))
                    tiles.append(gs)
                gsets.append(tiles)
            def emit_proj_chunk(prev, ec):
                # one output-projection chunk of the PREVIOUS iteration,
                # woven into the current attention phase as PE filler
                pjt_p, gi_p, bp2_p = prev
                pps = psa.tile([128, 2 * P], F32, tag="psa", name=f"pps{gi_p}_{bp2_p}_{ec}")
                for dc in range(DC):
                    nc.tensor.matmul(
                        pps,
                        wp[:, dc, 128 * ec:128 * (ec + 1)],
                        pjt_p[:, dc, :],
                        start=(dc == 0),
                        stop=(dc == DC - 1),
                    )
                ot = otp.tile([128, 2 * P], F32, tag="ot", name=f"ot{gi_p}_{bp2_p}_{ec}")
                nc.vector.tensor_scalar_add(ot, pps, bias[:, ec:ec + 1])
                for b2 in range(2):
                    nc.sync.dma_start(
                        out=o_d[2 * bp2_p + b2, 128 * ec:128 * (ec + 1), gi_p, :],
                        in_=ot[:, P * b2:P * (b2 + 1)],
                    )

            # Fully software-pipelined steady state: iteration t's QKV
            # chunk-groups are woven between iteration t-1's attention
            # heads so every engine (PE matmuls, ACT exp, DVE casts/recips,
            # GPSIMD mask) sees a uniform instruction stream — no
            # low-duty phase to re-throttle the PE clock.
            NT = GC * 2  # iterations: (gi, bp2) pairs

            def make_qkv_groups(t):
                gi, bp2 = divmod(t, 2)
                if t % 2 == 0:
                    mkt = maskp.tile([128, 2, P], BF16, tag="mk", name=f"mk{gi}")
                    for kc in range(2):
                        nc.sync.dma_start(
                            out=mkt[:, kc, :], in_=mt_d[gi, 128 * kc:128 * (kc + 1), :]
                        )
                    make_qkv_groups.mk = mkt
                mkt = make_qkv_groups.mk
                xt = xp.tile([128, DC, 2 * P], BF16, tag="xt", name=f"xt{t}")
                for b2 in range(2):
                    b = 2 * bp2 + b2
                    for dc in range(DC):
                        nc.sync.dma_start(
                            out=xt[:, dc, P * b2:P * (b2 + 1)],
                            in_=x_d[b, 128 * dc:128 * (dc + 1), gi, :],
                        )
                qt = qkvp.tile([128, DC, 2 * P], BF16, tag="qt", bufs=2, name=f"qt{t}")
                kt = qkvp.tile([128, DC, 2 * P], BF16, tag="kt", bufs=2, name=f"kt{t}")
                vt = qkvp.tile([128, 2, 2, H, HD + 1], BF16, tag="vt", bufs=2, name=f"vt{t}")
                nc.sync.dma_start(
                    out=vt[:, :, :, :, HD],
                    in_=on_d[:, 0:48].rearrange("p (a b h) -> p a b h", a=2, b=2),
                )

                groups = []

                def qk_group(c):
                    def emit():
                        ps = psa.tile([128, 2 * P], F32, tag="psa", name=f"ps{t}_{c}")
                        for dc in range(DC):
                            nc.tensor.matmul(
                                ps,
                                wq[:, dc, 128 * c:128 * (c + 1)],
                                xt[:, dc, :],
                                start=(dc == 0),
                                stop=(dc == DC - 1),
                            )
                        dst = qt if c < DC else kt
                        nc.vector.tensor_copy(dst[:, c % DC, :], ps)
                    return emit

                def v_group(b2, tkc, nh):
                    def emit():
                        psv = psa.tile([128, 384], F32, tag="psa", name=f"psv{t}_{b2}_{tkc}_{nh}")
                        t0 = P * b2 + 128 * tkc
                        for dc in range(DC):
                            nc.tensor.matmul(
                                psv,
                                xt[:, dc, t0:t0 + 128],
                                wq[:, dc, 2 * D + 384 * nh:2 * D + 384 * (nh + 1)],
                                start=(dc == 0),
                                stop=(dc == DC - 1),
                            )
                        nc.scalar.copy(
                            vt[:, b2, tkc, 6 * nh:6 * (nh + 1), 0:HD],
                            psv[:, :].rearrange("p (h d) -> p h d", h=6),
                        )
                    return emit

                for c in range(2 * DC):
                    groups.append(qk_group(c))
                for b2 in range(2):
                    for tkc in range(2):
                        for nh in range(2):
                            groups.append(v_group(b2, tkc, nh))
                return groups, (qt, kt, vt, mkt, gi, bp2)

            def make_attn_heads(ctx, t):
                qt, kt, vt, mkt, gi, bp2 = ctx
                pjt = pjp.tile([128, DC, 2 * P], BF16, tag="pjt", name=f"pjt{gi}_{bp2}")
                ats = [
                    attp.tile([128, DC, P], BF16, tag=f"at{b2}", name=f"at{gi}{bp2}{b2}")
                    for b2 in range(2)
                ]
                gset = gsets[t % 2]
                grp2 = [[gset[(3 * b2 + j)] for j in range(3)] for b2 in range(2)]
                ops2 = [None, None]
                heads = []

                def head_body(h, b2):
                    c, half = divmod(h, 2)
                    r0 = 64 * half
                    grow = 32 * (h % 4)

                    def emit():
                        at = ats[b2]
                        gt = grp2[b2][h // 4]
                        sps = psa.tile([128, 2 * P], F32, tag="psa", name=f"sps{h}_{b2}")
                        for kc in range(2):
                            nc.tensor.matmul(
                                sps[:, P * kc:P * (kc + 1)],
                                kt[r0:r0 + 64, c, P * b2 + 128 * kc:P * b2 + 128 * (kc + 1)],
                                qt[r0:r0 + 64, c, P * b2:P * (b2 + 1)],
                                start=True,
                                stop=True,
                            )
                        pt = ptp.tile([128, 2 * P], BF16, tag="pt", name=f"pt{h}_{b2}")
                        nc.scalar.activation(pt, sps, AF.Exp, scale=SCALE)
                        nc.gpsimd.tensor_mul(
                            pt, pt, mkt[:, :, :].rearrange("p a q -> p (a q)")
                        )
                        if half == 0:
                            ops2[b2] = psb.tile(
                                [HD + 1, 2 * P], F32, tag="psb", name=f"ops2_{b2}"
                            )
                        ops = ops2[b2][:, P * half:P * (half + 1)]
                        for kc in range(2):
                            nc.tensor.matmul(
                                ops,
                                vt[:, b2, kc, h, :],
                                pt[:, P * kc:P * (kc + 1)],
                                start=(kc == 0),
                                stop=(kc == 1),
                            )
                        nc.scalar.copy(gt[grow:grow + 1, :], ops[HD:HD + 1, :])
                        nc.vector.tensor_copy(at[r0:r0 + 64, c, :], ops[0:HD, :])
                    return emit

                for h in range(H):
                    for b2 in range(2):
                        heads.append(head_body(h, b2))

                def tail_piece(b2, grp):
                    def emit():
                        gs = grp2[b2][grp]
                        rs = rscp.tile([97, P], F32R, tag="rs", name=f"rs{b2}_{grp}")
                        nc.vector.reciprocal(rs, gs)
                        for pr in range(2):
                            c = 2 * grp + pr
                            Rps = psb.tile([128, P], F32, tag="psb", name=f"Rps{b2}_{grp}_{pr}")
                            nc.tensor.matmul(
                                Rps,
                                pairc[64 * pr:64 * pr + 33, :],
                                rs[64 * pr:64 * pr + 33, :],
                                start=True,
                                stop=True,
                            )
                            nc.vector.tensor_mul(
                                pjt[:, c, P * b2:P * (b2 + 1)],
                                ats[b2][:, c, :],
                                Rps,
                            )
                    return emit

                tailp = [tail_piece(b2, grp) for b2 in range(2) for grp in range(3)]
                return heads, tailp, (pjt, gi, bp2)

            # three-deep pipeline: pass t emits heads(t-1), qkv(t), and
            # the normalization tails + output projection of t-2, all
            # interleaved slot-by-slot
            hb = None  # bundle of iteration t-1 (heads pending)
            tb = None  # bundle of iteration t-2 (tails/proj pending)
            for t in range(NT + 2):
                groups = []
                ctx = None
                if t < NT:
                    groups, ctx = make_qkv_groups(t)
                heads = hb[0] if hb else []
                tailp = tb[1] if tb else []
                prev = tb[2] if tb else None
                gq = 0
                for s in range(24):
                    if s < len(heads):
                        heads[s]()
                    if s % 3 == 0 and s // 3 < len(tailp):
                        tailp[s // 3]()
                    if prev is not None and s >= 18:
                        emit_proj_chunk(prev, s - 18)
                    want = (s + 1) * len(groups) // 24
                    while gq < want:
                        groups[gq]()
                        gq += 1
                tb = hb
                hb = make_attn_heads(ctx, t) if t < NT else None
    _split_excess_waits(nc)
    return nc


def _prep_in_maps(x, mask, w_qkv, w_proj, b_proj):
    x = np.ascontiguousarray(x).astype(ml_dtypes.bfloat16)
    maskT = np.ascontiguousarray(
        np.transpose(mask, (0, 2, 1)).astype(ml_dtypes.bfloat16)
    )  # [g, key, query], exact 0/1 in bf16
    wqkvT = np.ascontiguousarray(w_qkv.T).astype(ml_dtypes.bfloat16)
    wprojT = np.ascontiguousarray(w_proj.T).astype(ml_dtypes.bfloat16)
    b_proj = np.ascontiguousarray(b_proj, dtype=np.float32)
    pair_c = np.zeros((97, 128), dtype=np.float32)
    pair_c[0, 0:64] = 1.0
    pair_c[32, 64:128] = 1.0
    pair_c[64, 0:64] = 1.0
    pair_c[96, 64:128] = 1.0
    in_maps = []
    for i in range(N_CORES):
        gs = slice(i * GC, (i + 1) * GC)
        in_maps.append(
            {
                "x": np.ascontiguousarray(x[:, :, gs, :]),
                "maskT": np.ascontiguousarray(maskT[gs]),
                "wqkvT": wqkvT,
                "wprojT": wprojT,
                "bproj": b_proj,
                "ones_c": np.ones((128, 64), dtype=ml_dtypes.bfloat16),
                "pair_c": pair_c,
                "rones": np.ones((97, P), dtype=np.float32),
            }
        )
    return in_maps


def _run(inputs, trace=False):
    nc = build_nc()
    in_maps = _prep_in_maps(
        inputs["x"], inputs["mask"], inputs["w_qkv"], inputs["w_proj"], inputs["b_proj"]
    )
    res = run_bass_kernel_spmd(nc, in_maps, list(range(N_CORES)), trace=trace)
    out = np.concatenate([res.results[i]["out"] for i in range(N_CORES)], axis=2)
    return out.astype(np.float32), res


def kernel(x, mask, w_qkv, w_proj, b_proj):
    out, _ = _run(
        {"x": x, "mask": mask, "w_qkv": w_qkv, "w_proj": w_proj, "b_proj": b_proj}
    )
    return out
